# revision 1
# baseline (speedup 1.0000x reference)
"""Trainium2 Bass kernel for nn_CardaicCircleNet_78675210928495.

Strategy: pure batch data-parallelism - 8 images, one per NeuronCore.
Per core the full forward pass runs on-chip:
  - convs as shifted matmuls (channels on partitions, spatial free),
    accumulating in PSUM; fp16 operands, fp32 accumulate; 1024-col chunks
  - input normalization computed on-chip; conv1 via flat im2col built
    through a DRAM bounce of the normalized padded image
  - conv1 / dw3 write both the plain and y-shifted (dy-pair) copies of
    their output directly (stationary columns duplicated), removing the
    serial SBUF shift DMAs
  - big conv weights streamed as fine-grained [128,25,128] slices with
    deep prefetch on the scalar DMA queue
  - soft rasterizer: per-edge signed distance is affine in pixel coords ->
    one [2,128]x[2,384] fp32r matmul per face, min/max on DVE with some
    faces offloaded to GpSimd via an ACT fp16 copy; renders interleaved
    with decoder conv chunks so DVE work hides under PE work
  - grid_sample as separable bilinear hat weights -> matmul over rows +
    masked reduce over cols
"""
import os
import sys

for _p in ("/opt/trn_rl_repo", "/root/.axon_site/_ro/trn_rl_repo"):
    if os.path.isdir(_p) and _p not in sys.path:
        sys.path.insert(0, _p)

import numpy as np

IMG = 128
N_FACES = 32
V = 33
CP0 = 16
SHARP = 128.0
ITER = 3
N_CORES = 8

_CACHE = {}

# (name, shape) of consts packed into the fp32 / fp16 blobs, in order
_F32SPEC = [('eb1_2', (128, 1)), ('eb2', (128, 1)), ('eb3', (128, 2)),
            ('eb4', (128, 4)),
            ('cb1', (128, 2)), ('cb2', (128, 1)), ('lb1', (1, 400)),
            ('lb2', (1, 200)), ('lb3', (1, 6)), ('db1', (128, 2)),
            ('db2', (128, 1)), ('db3_2', (128, 1)), ('dbo', (4, 1)),
            ('cst_xs128', (128, 128)), ('cst_ly', (2, 128)),
            ('cst_iotay', (128, 1)), ('cst_iotax33', (33, 128)),
            ('cst_onecol', (128, 1)), ('cst_u64', (64, 128)),
            ('cst_nodes1', (33, 2)), ('cst_nodes2', (33, 2)),
            ('cst_g0', (33, 96)), ('cst_g1', (33, 96)),
            ('cst_w2m', (33, 1)), ('cst_w0m', (33, 1))]
_F16SPEC = [('w2P', (128, 15, 128)), ('cw2T', (128, 2, 128)),
            ('lw2T', (100, 4, 200)), ('lw3T', (100, 2, 6))]

# weight stream: slice s is [128, 25, 128] fp16; order of consumption
# w3(2) w4(4x2) cw1(2x4) lw1(2) dw1(2x6) dw2(3) dw3a(1) dw3b+dwo(1)
N_STREAM = 2 + 8 + 8 + 2 + 12 + 3 + 1 + 1


def _blob_offsets(spec):
    off = {}
    c = 0
    for nm, sh in spec:
        w = 1
        for s in sh[1:]:
            w *= s
        off[nm] = (c, sh)
        c += w
    return off, c


# ---------------------------------------------------------------------------
# host-side constant / weight preparation (layout only, cached)
# ---------------------------------------------------------------------------

def _circles_np():
    th = 2.0 * np.pi * np.arange(N_FACES) / N_FACES
    ring = np.stack([np.cos(th), np.sin(th)], 1)
    nodes1 = np.vstack([0.5 * ring, [[0.0, 0.0]]]).astype(np.float32)
    nodes2 = np.vstack([0.3 * ring + [0.1, 0.0], [[0.1, 0.0]]]).astype(np.float32)
    faces = np.stack([np.arange(N_FACES), (np.arange(N_FACES) + 1) % N_FACES,
                      np.full(N_FACES, N_FACES)], 1)
    return nodes1, nodes2, faces


def _conv_wT(w, icb_count, ocb, oc_per_blk=128):
    """w: (OC, IC, 5, 5) -> [128, icb_count, 25, oc_per_blk] fp16 for ocb slice."""
    OC, IC = w.shape[:2]
    out = np.zeros((128, icb_count, 25, oc_per_blk), np.float16)
    for icb in range(icb_count):
        ic0 = icb * 128
        icn = min(128, IC - ic0)
        blk = w[ocb * oc_per_blk:(ocb + 1) * oc_per_blk, ic0:ic0 + icn]
        out[:icn, icb] = blk.transpose(1, 2, 3, 0).reshape(icn, 25, -1).astype(np.float16)
    return out


def _upmat64():
    """U[iny=64, outy=128] fp32: bilinear x2 upsample with edge clamp (lhsT)."""
    U = np.zeros((64, 128), np.float32)
    for j in range(64):
        jm = max(j - 1, 0)
        jp = min(j + 1, 63)
        U[jm, 2 * j] += 0.25
        U[j, 2 * j] += 0.75
        U[j, 2 * j + 1] += 0.75
        U[jp, 2 * j + 1] += 0.25
    return U


def _pair_pack(wT64, oc):
    """wT64: [64, 25, oc] -> [128, 15, oc]: taps (dy_lo in 0,2,4) x dx;
    rows 64-127 = dy_lo+1 tap (zero when dy_lo==4)."""
    out = np.zeros((128, 15, oc), np.float16)
    t = 0
    for dy_lo in (0, 2, 4):
        for dx in range(5):
            out[0:64, t] = wT64[:, dy_lo * 5 + dx]
            if dy_lo + 1 <= 4:
                out[64:128, t] = wT64[:, (dy_lo + 1) * 5 + dx]
            t += 1
    return out


def _prep_host(inputs):
    p = {k: np.asarray(v) for k, v in inputs.items()}
    d = {}
    # conv1 stationary: [25 taps, 128] = oc duplicated twice (plain+shifted)
    w1T = p['ew1'][:, 0].transpose(1, 2, 0).reshape(25, 64)
    d['w1P'] = np.concatenate([w1T, w1T], axis=1).astype(np.float16)  # (25,128)

    d['w2P'] = _pair_pack(_conv_wT(p['ew2'], 1, 0)[:64, 0], 128)   # [128, 15, 128]
    cw2 = p['cw2'][:, :, 0, 0]                              # (128, 256)
    d['cw2T'] = np.stack([cw2[:, k * 128:(k + 1) * 128].T for k in range(2)], 1).astype(np.float16)
    d['lw2T'] = p['lw2'].reshape(4, 100, 200).transpose(1, 0, 2).astype(np.float16)
    d['lw3T'] = p['lw3'].reshape(2, 100, 6).transpose(1, 0, 2).astype(np.float16)
    # dw3: duplicate output columns (plain + y-shifted copies of u3)
    dw3 = _conv_wT(p['dw3'], 2, 0, 64)                      # [128, 2, 25, 64]
    dw3a2 = np.concatenate([dw3[:, 0], dw3[:, 0]], axis=-1)               # [128,25,128]
    b = _pair_pack(dw3[:64, 1], 64)                                       # [128,15,64]
    dw3bP2 = np.concatenate([b, b], axis=-1)                              # [128,15,128]
    dwoP = _pair_pack(_conv_wT(p['dwo'], 1, 0, 4)[:64, 0], 4)             # [128, 15, 4]

    # ---- weight stream slices [128, S, 25*128] fp16, consumption order ----
    SL = 25 * 128
    slices = []
    for ocb in range(2):
        slices.append(_conv_wT(p['ew3'], 1, ocb)[:, 0].reshape(128, SL))
    for ocb in range(4):
        w4 = _conv_wT(p['ew4'], 2, ocb)                          # [128,2,25,128]
        slices.append(w4[:, 0].reshape(128, SL))
        slices.append(w4[:, 1].reshape(128, SL))
    for ocb in range(2):
        cw1 = _conv_wT(p['cw1'], 4, ocb)                         # [128,4,25,128]
        for icb in range(4):
            slices.append(cw1[:, icb].reshape(128, SL))
    lw1 = p['lw1'].reshape(128, 16, 400).astype(np.float16)
    slices.append(lw1[:, 0:8].reshape(128, SL))
    slices.append(lw1[:, 8:16].reshape(128, SL))
    for ocb in range(2):
        dw1 = _conv_wT(p['dw1'], 6, ocb)                         # [128,6,25,128]
        for ich in range(6):
            slices.append(dw1[:, ich].reshape(128, SL))
    dw2 = _conv_wT(p['dw2'], 3, 0)                               # [128,3,25,128]
    for icb in range(3):
        slices.append(dw2[:, icb].reshape(128, SL))
    slices.append(dw3a2.reshape(128, SL))
    last = np.zeros((128, SL), np.float16)
    last[:, :15 * 128] = dw3bP2.reshape(128, 15 * 128)
    last[:, 15 * 128:15 * 128 + 60] = dwoP.reshape(128, 60)
    slices.append(last)
    assert len(slices) == N_STREAM
    d['wstream'] = np.stack(slices, 1)

    # biases fp32
    eb1 = p['eb1'].reshape(64, 1).astype(np.float32)
    d['eb1_2'] = np.concatenate([eb1, eb1], axis=0)                   # [128,1]
    d['eb2'] = p['eb2'].reshape(128, 1).astype(np.float32)
    d['eb3'] = p['eb3'].reshape(2, 128).T.copy().astype(np.float32)   # [128, 2]
    d['eb4'] = p['eb4'].reshape(4, 128).T.copy().astype(np.float32)   # [128, 4]
    d['cb1'] = p['cb1'].reshape(2, 128).T.copy().astype(np.float32)
    d['cb2'] = p['cb2'].reshape(128, 1).astype(np.float32)
    d['lb1'] = p['lb1'].reshape(1, 400).astype(np.float32)
    d['lb2'] = p['lb2'].reshape(1, 200).astype(np.float32)
    d['lb3'] = p['lb3'].reshape(1, 6).astype(np.float32)
    d['db1'] = p['db1'].reshape(2, 128).T.copy().astype(np.float32)
    d['db2'] = p['db2'].reshape(128, 1).astype(np.float32)
    db3 = p['db3'].reshape(64, 1).astype(np.float32)
    d['db3_2'] = np.concatenate([db3, db3], axis=0)                   # [128,1]
    d['dbo'] = p['dbo'].reshape(4, 1).astype(np.float32)
    # constants
    xs = ((np.arange(IMG) + 0.5) * (2.0 / IMG) - 1.0).astype(np.float32)
    ys = (1.0 - (np.arange(IMG) + 0.5) * (2.0 / IMG)).astype(np.float32)
    d['cst_xs128'] = np.broadcast_to(xs, (128, 128)).copy()
    d['cst_ly'] = np.stack([np.ones(128, np.float32), ys], 0)         # [2, 128]
    d['cst_ones'] = np.ones((1, 128), np.float32)
    d['cst_iotay'] = np.arange(128, dtype=np.float32).reshape(128, 1)
    d['cst_iotax33'] = np.broadcast_to(np.arange(128, dtype=np.float32), (33, 128)).copy()
    d['cst_onecol'] = np.ones((128, 1), np.float32)
    d['cst_negones2'] = np.full((2, 1), -1.0, np.float32)
    d['cst_u64'] = _upmat64()
    nodes1, nodes2, faces = _circles_np()
    d['cst_nodes1'] = nodes1
    d['cst_nodes2'] = nodes2
    G0 = np.zeros((33, 96), np.float32)
    G1 = np.zeros((33, 96), np.float32)
    nxt = np.roll(np.arange(3), -1)
    for f in range(N_FACES):
        for j in range(3):
            G0[faces[f][j], f * 3 + j] = 1.0
            G1[faces[f][nxt[j]], f * 3 + j] = 1.0
    d['cst_g0'] = G0
    d['cst_g1'] = G1
    idx = np.arange(V)
    d['cst_w2m'] = (idx <= CP0).astype(np.float32).reshape(33, 1)
    d['cst_w0m'] = ((idx >= CP0).astype(np.float32)
                    + (idx == V - 1).astype(np.float32)).reshape(33, 1)

    off32, w32 = _blob_offsets(_F32SPEC)
    blob32 = np.zeros((128, w32), np.float32)
    for nm, sh in _F32SPEC:
        a = d[nm]
        c0, _ = off32[nm]
        blob32[:a.shape[0], c0:c0 + int(np.prod(sh[1:]))] = a.reshape(a.shape[0], -1)
        del d[nm]
    off16, w16 = _blob_offsets(_F16SPEC)
    blob16 = np.zeros((128, w16), np.float16)
    for nm, sh in _F16SPEC:
        a = d[nm]
        c0, _ = off16[nm]
        blob16[:a.shape[0], c0:c0 + int(np.prod(sh[1:]))] = a.reshape(a.shape[0], -1)
        del d[nm]
    d['blob32'] = blob32
    d['blob16'] = blob16
    return d


# ---------------------------------------------------------------------------
# device program
# ---------------------------------------------------------------------------

def _build_program(debug=False):
    import concourse.bass as bass
    import concourse.tile as tile
    from concourse import mybir, bacc
    from concourse.masks import make_identity

    F32 = mybir.dt.float32
    F16 = mybir.dt.float16

    nc = bacc.Bacc("TRN2", num_devices=N_CORES, debug=False)

    din = {}
    def dt_in(name, shape, dtype=F32):
        din[name] = nc.dram_tensor(name, list(shape), dtype, kind="ExternalInput")
        return din[name]

    dt_in("img", (128, 128))
    dt_in("w1P", (25, 128), F16)
    dt_in("wstream", (128, N_STREAM, 25 * 128), F16)
    _o32, _w32 = _blob_offsets(_F32SPEC)
    _o16, _w16 = _blob_offsets(_F16SPEC)
    dt_in("blob32", (128, _w32))
    dt_in("blob16", (128, _w16), F16)
    for nm, sh in [("cst_ones", (1, 128)), ("cst_negones2", (2, 1))]:
        dt_in(nm, sh)

    out_d = nc.dram_tensor("out", [4, 128, 128], F32, kind="ExternalOutput")
    dbg = {}
    if debug:
        for nm, sh, dt_ in [("dbg_f1", (128, 68, 68), F16),
                            ("dbg_f4", (128, 4, 12, 12), F16),
                            ("dbg_cb", (128, 16), F16), ("dbg_aff", (1, 6), F32),
                            ("dbg_u3", (128, 68, 68), F16),
                            ("dbg_disp", (128, 4, 128), F32),
                            ("dbg_n1", (33, 2), F32), ("dbg_n2", (33, 2), F32)]:
            dbg[nm] = nc.dram_tensor(nm, list(sh), dt_, kind="ExternalOutput")

    with tile.TileContext(nc) as tc:
        _emit(nc, tc, tile, bass, mybir, din, out_d, dbg, make_identity, debug)

    nc.compile()
    return nc


def _emit(nc, tc, tile, bass, mybir, din, out_d, dbg, make_identity, debug):
    F32 = mybir.dt.float32
    F32R = mybir.dt.float32r
    F16 = mybir.dt.float16
    AF = mybir.ActivationFunctionType
    ALU = mybir.AluOpType
    AX = mybir.AxisListType
    ts = bass.ts

    from contextlib import ExitStack
    ctx = ExitStack()

    consts = ctx.enter_context(tc.tile_pool(name="consts", bufs=1))
    feat = ctx.enter_context(tc.tile_pool(name="feat", bufs=1))
    chunks = ctx.enter_context(tc.tile_pool(name="chunks", bufs=3))
    temps = ctx.enter_context(tc.tile_pool(name="temps", bufs=2))
    small = ctx.enter_context(tc.tile_pool(name="small", bufs=2))
    nodes_p = ctx.enter_context(tc.tile_pool(name="nodes", bufs=10))
    psum_s = ctx.enter_context(tc.tile_pool(name="psum_s", bufs=2, space="PSUM"))
    dram = ctx.enter_context(tc.tile_pool(name="dram", bufs=1, space="DRAM"))
    rendp = ctx.enter_context(tc.tile_pool(name="rendp", bufs=2))
    grpp = ctx.enter_context(tc.tile_pool(name="grpp", bufs=2))

    _o32, _ = _blob_offsets(_F32SPEC)
    _o16, _ = _blob_offsets(_F16SPEC)

    # ---- earliest DMAs ----------------------------------------------------
    t_img = small.tile([128, 128], F32, tag="timg")
    nc.sync.dma_start(t_img[:], din["img"].ap())
    w1P = consts.tile([25, 128], F16, tag="w1P")
    nc.sync.dma_start(w1P[:], din["w1P"].ap())
    NEG2 = consts.tile([2, 1], F32, tag="neg2")
    nc.sync.dma_start(NEG2[:], din["cst_negones2"].ap())
    ONES = consts.tile([1, 128], F32, tag="ones")
    nc.sync.dma_start(ONES[:], din["cst_ones"].ap())
    B32 = consts.tile([128, _blob_offsets(_F32SPEC)[1]], F32, tag="b32")
    nc.scalar.dma_start(B32[:], din["blob32"].ap())
    B16 = consts.tile([128, _blob_offsets(_F16SPEC)[1]], F16, tag="b16")
    nc.scalar.dma_start(B16[:], din["blob16"].ap())

    def c32(nm):
        c0, sh = _o32[nm]
        w = 1
        for s in sh[1:]:
            w *= s
        ap = B32[0:sh[0], c0:c0 + w]
        if len(sh) == 3:
            ap = ap.rearrange("p (a b) -> p a b", a=sh[1])
        return ap

    def c16(nm):
        c0, sh = _o16[nm]
        w = 1
        for s in sh[1:]:
            w *= s
        ap = B16[0:sh[0], c0:c0 + w]
        if len(sh) == 3:
            ap = ap.rearrange("p (a b) -> p a b", a=sh[1])
        return ap

    w2P = c16("w2P"); cw2T = c16("cw2T"); lw2T = c16("lw2T")
    lw3T = c16("lw3T")
    eb1 = c32("eb1_2"); eb2 = c32("eb2"); eb3 = c32("eb3"); eb4 = c32("eb4")
    cb1 = c32("cb1"); cb2 = c32("cb2")
    lb1 = c32("lb1"); lb2 = c32("lb2"); lb3 = c32("lb3")
    db1 = c32("db1"); db2 = c32("db2"); db3 = c32("db3_2"); dbo = c32("dbo")
    XS = c32("cst_xs128"); LY = c32("cst_ly"); IOTAY = c32("cst_iotay")
    IOTAX33 = c32("cst_iotax33"); ONECOL = c32("cst_onecol")
    U64 = c32("cst_u64"); NODES1 = c32("cst_nodes1"); NODES2 = c32("cst_nodes2")
    G0 = c32("cst_g0"); G1 = c32("cst_g1")
    W2M = c32("cst_w2m"); W0M = c32("cst_w0m")
    IDENT = consts.tile([128, 128], F32, tag="ident")
    make_identity(nc, IDENT)

    # ---- persistent feature buffers (border-only zeroing) -----------------
    f1_pad = feat.tile([128, 68, 68], F16, tag="f1_pad")
    f2_pad = feat.tile([128, 36, 36], F16, tag="f2_pad")
    f3_pad = feat.tile([128, 2, 20, 20], F16, tag="f3_pad")
    f4_pad = feat.tile([128, 4, 12, 12], F16, tag="f4_pad")
    up4_pad = feat.tile([128, 4, 20, 20], F16, tag="up4_pad")
    u1_pad = feat.tile([128, 2, 20, 20], F16, tag="u1_pad")
    u1up_pad = feat.tile([128, 2, 36, 36], F16, tag="u1up_pad")
    u2_pad = feat.tile([128, 36, 36], F16, tag="u2_pad")
    u2up_pad = feat.tile([128, 68, 68], F16, tag="u2up_pad")
    u3_pad = feat.tile([128, 68, 68], F16, tag="u3_pad")
    disp_sb = feat.tile([128, 4, 128], F32, tag="disp")

    def zero_borders(t, nblk, H, eng):
        # zero the 2-px border ring of each [128, H, H] block
        if nblk == 1:
            v = t.rearrange("p (b y) x -> p b y x", b=1)
        else:
            v = t
        eng.memset(v[:, :, 0:2, :], 0.0)
        eng.memset(v[:, :, H - 2:H, :], 0.0)
        eng.memset(v[:, :, 2:H - 2, 0:2], 0.0)
        eng.memset(v[:, :, 2:H - 2, H - 2:H], 0.0)

    zero_borders(f1_pad, 1, 68, nc.gpsimd)
    zero_borders(f2_pad, 1, 36, nc.gpsimd)
    zero_borders(f3_pad, 2, 20, nc.gpsimd)
    zero_borders(f4_pad, 4, 12, nc.gpsimd)
    zero_borders(up4_pad, 4, 20, nc.gpsimd)
    zero_borders(u1_pad, 2, 20, nc.gpsimd)
    zero_borders(u1up_pad, 2, 36, nc.gpsimd)
    zero_borders(u2_pad, 1, 36, nc.gpsimd)
    zero_borders(u2up_pad, 1, 68, nc.gpsimd)
    zero_borders(u3_pad, 1, 68, nc.gpsimd)
    # f1/u3 shifted-copy rows outside interior writes (see conv1/dw3)
    nc.gpsimd.memset(f1_pad[64:128, 65:68, :], 0.0)
    nc.gpsimd.memset(u3_pad[64:128, 65:68, :], 0.0)

    # ---- stage 0: min/max -> scale/shift, normalized padded image ---------
    r2 = small.tile([128, 2], F32, tag="r2")
    nc.vector.tensor_reduce(r2[:, 0:1], t_img[:], AX.X, ALU.min)
    nc.vector.tensor_reduce(r2[:, 1:2], t_img[:], AX.X, ALU.max, negate=True)
    tr2 = psum_s.tile([2, 128], F32, tag="sps")
    nc.tensor.transpose(tr2[:], r2[:], IDENT[:])
    rmm = small.tile([2, 1], F32, tag="rmm")
    nc.vector.tensor_reduce(rmm[:], tr2[:], AX.X, ALU.min)   # [mn, -mx]
    pden = psum_s.tile([1, 1], F32, tag="sps")
    nc.tensor.matmul(pden[:], NEG2[:], rmm[:], start=True, stop=True)  # mx-mn
    den = small.tile([1, 1], F32, tag="den")
    nc.vector.tensor_scalar_add(den[:], pden[:], 0.01)
    sc = small.tile([1, 1], F32, tag="sc")
    nc.vector.reciprocal(sc[:], den[:])
    shp = small.tile([1, 1], F32, tag="shp")
    nc.vector.tensor_tensor(shp[:], rmm[0:1, :], sc[:], ALU.mult)   # mn*sc
    scsh = small.tile([1, 2], F32, tag="scsh")
    nc.vector.tensor_copy(scsh[0:1, 0:1], sc[:])
    nc.vector.tensor_scalar_mul(scsh[0:1, 1:2], shp[:], -1.0)
    pbc = psum_s.tile([128, 2], F32, tag="sps")
    nc.tensor.matmul(pbc[:], ONES[0:1, :], scsh[:], start=True, stop=True)
    bc = small.tile([128, 2], F32, tag="bc")
    nc.scalar.copy(bc[:], pbc[:])
    # normalized padded image rows (x-padding included), fp16
    nimg = small.tile([128, 132], F16, tag="nimg")
    nc.vector.memset(nimg[:, 0:2], 0.0)
    nc.vector.memset(nimg[:, 130:132], 0.0)
    nc.vector.tensor_scalar(nimg[:, 2:130], t_img[:], bc[:, 0:1], bc[:, 1:2],
                            ALU.mult, ALU.add)
    zrow = small.tile([2, 132], F16, tag="zrow")
    nc.vector.memset(zrow[:], 0.0)
    pad_scr = dram.tile([132, 132], F16, tag="pad_scr")
    nc.sync.dma_start(pad_scr[0:2], zrow[:])
    nc.sync.dma_start(pad_scr[130:132], zrow[:])
    nc.sync.dma_start(pad_scr[2:130], nimg[:])

    PAIR_TAPS = [(dy_lo, dx) for dy_lo in (0, 2, 4) for dx in range(5)]

    def relu_pool(ps, oc, nrows, W_out, bias_ap, dst_ap):
        """relu(ps+bias) -> fp16 -> 2x2 maxpool -> dst_ap [oc, nrows/2, W_out/2]."""
        ct = chunks.tile([oc, nrows, W_out], F16, tag="ct")
        nc.scalar.activation(ct.rearrange("p a b -> p (a b)"), ps,
                             AF.Relu, bias=bias_ap, scale=1.0)
        mr = temps.tile([oc, nrows // 2, W_out], F16, tag="mr")
        nc.vector.tensor_tensor(mr[:], ct[:, 0::2, :], ct[:, 1::2, :], ALU.max)
        nc.vector.tensor_tensor(dst_ap, mr[:, :, 0::2], mr[:, :, 1::2], ALU.max)

    # ---- weight stream ----------------------------------------------------
    upt = ctx.enter_context(tc.tile_pool(name="upt", bufs=1))
    wpool = ctx.enter_context(tc.tile_pool(name="wpool", bufs=5))
    _snext = [0]

    def wslice():
        s = _snext[0]
        _snext[0] += 1
        t = wpool.tile([128, 25, 128], F16, tag="ws")
        nc.scalar.dma_start(t.rearrange("p a b -> p (a b)"),
                            din["wstream"].ap()[:, s])
        return t

    # ---- conv1 + conv2 + conv3 (shared psum pool, 1024-col chunks) --------
    with tc.tile_pool(name="i2c", bufs=1) as i2cp, \
         tc.tile_pool(name="psum_c", bufs=3, space="PSUM") as psum_c:
        imgp = pad_scr[:].rearrange("a b -> (a b)")
        I2Cv = None
        for c in range(32):
            if c % 16 == 0:
                # consumed flat range per partition is [0, 63*132+128); the
                # full 68*132 span would run the gather past the image end
                I2C = i2cp.tile([25, 68 * 132], F16, tag="i2c", name="I2C")
                slab_src = bass.AP(tensor=imgp.tensor,
                                   offset=imgp.offset + (64 * 132) * (c // 16),
                                   ap=[[132, 5], [1, 5], [1, 63 * 132 + 128]])
                nc.sync.dma_start(I2C[:, 0:63 * 132 + 128], slab_src)
                I2Cv = I2C.rearrange("p (y x) -> p y x", x=132)
            cc = c % 16
            ps = psum_c.tile([128, 512], F32, tag="cps")
            nc.tensor.matmul(ps.rearrange("p (a b) -> p a b", a=4),
                             w1P[:], I2Cv[:, 4 * cc:4 * cc + 4, 0:128],
                             start=True, stop=True)
            ct = chunks.tile([128, 4, 128], F16, tag="ct")
            nc.scalar.activation(ct.rearrange("p a b -> p (a b)"), ps[:],
                                 AF.Relu, bias=eb1[:], scale=1.0)
            mr = temps.tile([128, 2, 128], F16, tag="mr")
            nc.vector.tensor_tensor(mr[:], ct[:, 0::2, :], ct[:, 1::2, :], ALU.max)
            nc.vector.tensor_tensor(f1_pad[0:64, 2 + 2 * c:4 + 2 * c, 2:66],
                                    mr[0:64, :, 0::2], mr[0:64, :, 1::2], ALU.max)
            nc.vector.tensor_tensor(f1_pad[64:128, 1 + 2 * c:3 + 2 * c, 2:66],
                                    mr[64:128, :, 0::2], mr[64:128, :, 1::2], ALU.max)

        if debug:
            nc.sync.dma_start(dbg["dbg_f1"].ap(), f1_pad[:])

        # conv2: dy-pair packed, 8 chunks of 8 out rows
        for c in range(8):
            ps = psum_c.tile([128, 512], F32, tag="cps")
            psv = ps.rearrange("p (a b) -> p a b", a=8)
            for t, (dy_lo, dx) in enumerate(PAIR_TAPS):
                nc.tensor.matmul(psv, w2P[:, t, :],
                                 f1_pad[:, dy_lo + 8 * c:dy_lo + 8 * c + 8, dx:dx + 64],
                                 start=(t == 0), stop=(t == 14))
            relu_pool(ps[:], 128, 8, 64, eb2[:], f2_pad[:, 2 + 4 * c:6 + 4 * c, 2:34])

        # conv3: 2 chunks of 16 out rows per ocb
        w3s = [wslice(), wslice()]
        for c in range(2):
            for ocb in range(2):
                ps = psum_c.tile([128, 512], F32, tag="cps")
                psv = ps.rearrange("p (a b) -> p a b", a=16)
                for tap in range(25):
                    dy, dx = tap // 5, tap % 5
                    nc.tensor.matmul(psv, w3s[ocb][:, tap, :],
                                     f2_pad[:, dy + 16 * c:dy + 16 * c + 16, dx:dx + 32],
                                     start=(tap == 0), stop=(tap == 24))
                relu_pool(ps[:], 128, 16, 32, eb3[:, ocb:ocb + 1],
                          f3_pad[:, ocb, 2 + 8 * c:10 + 8 * c, 2:18])

    # ---- conv4 + cw1 + cw2 + FC ------------------------------------------
    def upsample2(src, dst_interior, P, nblk, H, W):
        up_t = upt.tile([P, nblk, 2 * H, W], F16, tag=f"up_t{H}")
        ta = upt.tile([P, nblk, H - 1, W], F16, tag=f"up_a{H}")
        nc.vector.tensor_copy(up_t[:, :, 0:1, :], src[:, :, 0:1, :])
        nc.vector.tensor_scalar_mul(ta[:], src[:, :, 0:H - 1, :], 1.0 / 3.0)
        nc.vector.tensor_tensor(ta[:], ta[:], src[:, :, 1:H, :], ALU.add)
        nc.vector.tensor_scalar_mul(up_t[:, :, 2:2 * H - 1:2, :], ta[:], 0.75)
        nc.vector.tensor_scalar_mul(ta[:], src[:, :, 1:H, :], 1.0 / 3.0)
        nc.vector.tensor_tensor(ta[:], ta[:], src[:, :, 0:H - 1, :], ALU.add)
        nc.vector.tensor_scalar_mul(up_t[:, :, 1:2 * H - 2:2, :], ta[:], 0.75)
        nc.vector.tensor_copy(up_t[:, :, 2 * H - 1:2 * H, :], src[:, :, H - 1:H, :])
        tb = upt.tile([P, nblk, 2 * H, W - 1], F16, tag=f"up_b{H}")
        nc.vector.tensor_copy(dst_interior[:, :, :, 0:1], up_t[:, :, :, 0:1])
        nc.vector.tensor_scalar_mul(tb[:], up_t[:, :, :, 0:W - 1], 1.0 / 3.0)
        nc.vector.tensor_tensor(tb[:], tb[:], up_t[:, :, :, 1:W], ALU.add)
        nc.vector.tensor_scalar_mul(dst_interior[:, :, :, 2:2 * W - 1:2], tb[:], 0.75)
        nc.vector.tensor_scalar_mul(tb[:], up_t[:, :, :, 1:W], 1.0 / 3.0)
        nc.vector.tensor_tensor(tb[:], tb[:], up_t[:, :, :, 0:W - 1], ALU.add)
        nc.vector.tensor_scalar_mul(dst_interior[:, :, :, 1:2 * W - 2:2], tb[:], 0.75)
        nc.vector.tensor_copy(dst_interior[:, :, :, 2 * W - 1:2 * W],
                              up_t[:, :, :, W - 1:W])

    with tc.tile_pool(name="psum_m", bufs=3, space="PSUM") as psum_m:
        # conv4
        for ocb in range(4):
            wa, wb = wslice(), wslice()
            ps = psum_m.tile([128, 256], F32, tag="mps")
            psv = ps.rearrange("p (a b) -> p a b", a=16)
            first = True
            for bi, w in enumerate((wa, wb)):
                for tap in range(25):
                    dy, dx = tap // 5, tap % 5
                    nc.tensor.matmul(psv, w[:, tap, :],
                                     f3_pad[:, bi, dy:dy + 16, dx:dx + 16],
                                     start=first, stop=(bi == 1 and tap == 24))
                    first = False
            relu_pool(ps[:], 128, 16, 16, eb4[:, ocb:ocb + 1],
                      f4_pad[:, ocb, 2:10, 2:10])
        if debug:
            nc.sync.dma_start(dbg["dbg_f4"].ap(), f4_pad[:])

        # up4 upsample early (DVE) so dw1 can start right after FC
        upsample2(f4_pad[:, :, 2:10, 2:10], up4_pad[:, :, 2:18, 2:18], 128, 4, 8, 8)

        # cw1
        ca = feat.tile([128, 2, 4, 4], F16, tag="ca")
        for ocb in range(2):
            ws4 = [wslice() for _ in range(4)]
            ps_full = psum_m.tile([128, 256], F32, tag="mps")
            ps = ps_full[:, 0:64]
            psv = ps.rearrange("p (a b) -> p a b", a=8)
            first = True
            for bi in range(4):
                for tap in range(25):
                    dy, dx = tap // 5, tap % 5
                    nc.tensor.matmul(psv, ws4[bi][:, tap, :],
                                     f4_pad[:, bi, dy:dy + 8, dx:dx + 8],
                                     start=first, stop=(bi == 3 and tap == 24))
                    first = False
            relu_pool(ps[:], 128, 8, 8, cb1[:, ocb:ocb + 1], ca[:, ocb])

        # cw2 1x1
        ps6 = psum_s.tile([128, 16], F32, tag="sps")
        caf = ca.rearrange("p b y x -> p b (y x)")
        for icb in range(2):
            nc.tensor.matmul(ps6[:], cw2T[:, icb, :], caf[:, icb, :],
                             start=(icb == 0), stop=(icb == 1))
        cbt = feat.tile([128, 16], F16, tag="cb")
        nc.scalar.activation(cbt[:], ps6[:], AF.Relu, bias=cb2[:], scale=1.0)
        if debug:
            nc.sync.dma_start(dbg["dbg_cb"].ap(), cbt[:])

        # FC head (lw1 arrives as two stream slices of 8x400)
        lw1a = wslice().rearrange("p a b -> p (a b)").rearrange(
            "p (a b) -> p a b", a=8)
        lw1b = wslice().rearrange("p a b -> p (a b)").rearrange(
            "p (a b) -> p a b", a=8)
        ps7 = psum_s.tile([1, 400], F32, tag="sps")
        for s in range(16):
            lw1s = lw1a if s < 8 else lw1b
            nc.tensor.matmul(ps7[:], cbt[:, s:s + 1], lw1s[:, s % 8, :],
                             start=(s == 0), stop=(s == 15))
        a1r = small.tile([1, 400], F32, tag="a1r")
        nc.vector.tensor_tensor(a1r[:], ps7[:], lb1[:], ALU.add)
        nc.vector.tensor_scalar_max(a1r[:], a1r[:], 0.0)
        a1c = small.tile([100, 4], F16, tag="a1c")
        for k in range(4):
            pt = psum_s.tile([100, 1], F32, tag="sps")
            nc.tensor.transpose(pt[:], a1r[0:1, ts(k, 100)], IDENT[0:1, 0:1])
            nc.scalar.copy(a1c[:, k:k + 1], pt[:])
        ps8 = psum_s.tile([1, 200], F32, tag="sps")
        for k in range(4):
            nc.tensor.matmul(ps8[:], a1c[:, k:k + 1], lw2T[:, k, :],
                             start=(k == 0), stop=(k == 3))
        a2r = small.tile([1, 200], F32, tag="a2r")
        nc.vector.tensor_tensor(a2r[:], ps8[:], lb2[:], ALU.add)
        nc.vector.tensor_scalar_max(a2r[:], a2r[:], 0.0)
        a2c = small.tile([100, 2], F16, tag="a2c")
        for k in range(2):
            pt = psum_s.tile([100, 1], F32, tag="sps")
            nc.tensor.transpose(pt[:], a2r[0:1, ts(k, 100)], IDENT[0:1, 0:1])
            nc.scalar.copy(a2c[:, k:k + 1], pt[:])
        ps9 = psum_s.tile([1, 6], F32, tag="sps")
        for k in range(2):
            nc.tensor.matmul(ps9[:], a2c[:, k:k + 1], lw3T[:, k, :],
                             start=(k == 0), stop=(k == 1))
        afz = small.tile([1, 6], F32, tag="afz")
        nc.vector.tensor_tensor(afz[:], ps9[:], lb3[:], ALU.add)
        aff = small.tile([1, 6], F32, tag="aff")
        nc.scalar.activation(aff[:], afz[:], AF.Tanh)
        if debug:
            nc.sync.dma_start(dbg["dbg_aff"].ap(), aff[:])

        # affine node transform
        paf = psum_s.tile([33, 6], F32, tag="sps")
        nc.tensor.matmul(paf[:], ONES[0:1, 0:33], aff[:], start=True, stop=True)
        affb = small.tile([33, 6], F32, tag="affb")
        nc.scalar.copy(affb[:], paf[:])

    def affine_nodes(nodes_const, tag):
        n = nodes_p.tile([33, 2], F32, tag=tag)
        u = temps.tile([33, 1], F32, tag="affu")
        v = temps.tile([33, 1], F32, tag="affv")
        nc.vector.tensor_scalar_mul(u[:], nodes_const[:, 0:1], affb[:, 0:1])
        nc.vector.tensor_scalar_mul(v[:], nodes_const[:, 1:2], affb[:, 3:4])
        nc.vector.tensor_tensor(n[:, 0:1], u[:], v[:], ALU.add)
        nc.vector.tensor_scalar_mul(u[:], nodes_const[:, 0:1], affb[:, 1:2])
        nc.vector.tensor_scalar_mul(v[:], nodes_const[:, 1:2], affb[:, 4:5])
        nc.vector.tensor_tensor(n[:, 1:2], u[:], v[:], ALU.add)
        return n

    n1 = affine_nodes(NODES1, "n1_0")
    n2 = affine_nodes(NODES2, "n2_0")

    LYr = consts.tile([2, 128], F32R, tag="lyr")
    nc.vector.tensor_copy(LYr[:], LY[:])

    # ---- renderer (setup / face-streaming split) --------------------------
    rend_scr = dram.tile([4, 96, 256], F32R, tag="rend_scr")
    macc = {}      # (rslot, path) -> (tile, first_flag_list)

    def render_setup(nodes_t, rslot):
        """Emit coefficient computation for one render; returns state."""
        rows = {}
        for nm, lhsT, G in (("v0x", nodes_t[:, 0:1], G0), ("v0y", nodes_t[:, 1:2], G0),
                            ("v1x", nodes_t[:, 0:1], G1), ("v1y", nodes_t[:, 1:2], G1)):
            pg = psum_s.tile([1, 96], F32, tag="sps")
            nc.tensor.matmul(pg[:], lhsT, G[:], start=True, stop=True)
            t = rendp.tile([1, 96], F32, tag=f"r_{nm}")
            nc.scalar.copy(t[:], pg[:])
            rows[nm] = t

        def op2(nm, i0, i1, op):
            t = rendp.tile([1, 96], F32, tag=f"r_{nm}")
            nc.vector.tensor_tensor(t[:], i0, i1, op)
            return t

        ex = op2("ex", rows["v1x"][:], rows["v0x"][:], ALU.subtract)
        ey = op2("ey", rows["v1y"][:], rows["v0y"][:], ALU.subtract)
        ex2 = op2("ex2", ex[:], ex[:], ALU.mult)
        ey2 = op2("ey2", ey[:], ey[:], ALU.mult)
        e2 = op2("e2", ex2[:], ey2[:], ALU.add)
        el = rendp.tile([1, 96], F32, tag="r_el")
        nc.scalar.activation(el[:], e2[:], AF.Sqrt)
        nc.vector.tensor_scalar_add(el[:], el[:], 1e-8)
        il = rendp.tile([1, 96], F32, tag="r_il")
        nc.vector.reciprocal(il[:], el[:])
        fx0 = rows["v0x"][0:1, 0::3]; fx1 = rows["v0x"][0:1, 1::3]; fx2 = rows["v0x"][0:1, 2::3]
        fy0 = rows["v0y"][0:1, 0::3]; fy1 = rows["v0y"][0:1, 1::3]; fy2 = rows["v0y"][0:1, 2::3]
        d10x = rendp.tile([1, 32], F32, tag="r_a1")
        nc.vector.tensor_tensor(d10x[:], fx1, fx0, ALU.subtract)
        d20y = rendp.tile([1, 32], F32, tag="r_a2")
        nc.vector.tensor_tensor(d20y[:], fy2, fy0, ALU.subtract)
        p1t = rendp.tile([1, 32], F32, tag="r_a3")
        nc.vector.tensor_tensor(p1t[:], d10x[:], d20y[:], ALU.mult)
        d10y = rendp.tile([1, 32], F32, tag="r_a4")
        nc.vector.tensor_tensor(d10y[:], fy1, fy0, ALU.subtract)
        d20x = rendp.tile([1, 32], F32, tag="r_a5")
        nc.vector.tensor_tensor(d20x[:], fx2, fx0, ALU.subtract)
        p2t = rendp.tile([1, 32], F32, tag="r_a6")
        nc.vector.tensor_tensor(p2t[:], d10y[:], d20x[:], ALU.mult)
        area = rendp.tile([1, 32], F32, tag="r_area")
        nc.vector.tensor_tensor(area[:], p1t[:], p2t[:], ALU.subtract)
        sg = rendp.tile([1, 32], F32, tag="r_sg")
        nc.scalar.activation(sg[:], area[:], AF.Sign)
        s96 = rendp.tile([1, 96], F32, tag="r_s96")
        for j in range(3):
            nc.vector.tensor_copy(s96[0:1, j::3], sg[:])
        m = rendp.tile([1, 96], F32, tag="r_m")
        nc.vector.tensor_tensor(m[:], s96[:], il[:], ALU.mult)
        nc.vector.tensor_scalar_mul(m[:], m[:], SHARP)
        mneg = rendp.tile([1, 96], F32, tag="r_mneg")
        nc.vector.tensor_scalar_mul(mneg[:], m[:], -1.0)
        acoef = op2("acoef", ey[:], mneg[:], ALU.mult)
        bcoef = op2("bcoef", ex[:], m[:], ALU.mult)
        cx = op2("cx", ey[:], rows["v0x"][:], ALU.mult)
        cy = op2("cy", ex[:], rows["v0y"][:], ALU.mult)
        cd = op2("cd", cx[:], cy[:], ALU.subtract)
        ccoef = op2("ccoef", cd[:], m[:], ALU.mult)
        pct = psum_s.tile([96, 3], F32, tag="sps")
        nc.tensor.transpose(pct[:, 0:1], acoef[:], IDENT[0:1, 0:1])
        nc.tensor.transpose(pct[:, 1:2], bcoef[:], IDENT[0:1, 0:1])
        nc.tensor.transpose(pct[:, 2:3], ccoef[:], IDENT[0:1, 0:1])
        acb = rendp.tile([96, 3], F32, tag="r_acb")
        nc.scalar.copy(acb[:], pct[:])
        RB = rendp.tile([96, 256], F32R, tag="r_RB")
        nc.vector.tensor_scalar(RB[:, 0:128], XS[0:96, :], acb[:, 0:1],
                                acb[:, 2:3], ALU.mult, ALU.add)
        nc.vector.tensor_scalar(RB[:, 128:256], XS[0:96, :], 0.0,
                                acb[:, 1:2], ALU.mult, ALU.add)
        nc.sync.dma_start(rend_scr[rslot], RB[:])
        maccD = feat.tile([128, 128], F32, tag=f"maccD{rslot}", name=f"maccD{rslot}")
        gmin = feat.tile([128, 8, 128], F16, tag=f"gmin{rslot}", name=f"gmin{rslot}")
        macc[(rslot, 'dve')] = [maccD, True]
        return {'rslot': rslot, 'grp': None, 'gmin': gmin}

    def render_faces(st, pD_pool, f0, f1, gp_mod=0):
        """Emit face matmuls + per-face min into gmin slots for faces
        [f0, f1); every 8th face collapses the slots into macc."""
        rslot = st['rslot']
        gmin = st['gmin']
        scr = rend_scr[rslot].rearrange("e c -> (e c)")
        for fi in range(f0, f1):
            g = fi // 4
            if st['grp'] is None or st['grp'][0] != g:
                grp2 = grpp.tile([2, 1536], F32R, tag="r_grp2")
                src = bass.AP(tensor=scr.tensor, offset=scr.offset + g * 12 * 256,
                              ap=[[128, 2], [256, 12], [1, 128]])
                nc.sync.dma_start(grp2.rearrange("p (e x) -> p e x", e=12), src)
                st['grp'] = (g, grp2)
            grp2 = st['grp'][1]
            fl = fi % 4
            pD = pD_pool.tile([128, 384], F32, tag="rpD")
            nc.tensor.matmul(pD[:], LYr[:], grp2[:, ts(fl, 384)],
                             start=True, stop=True)
            pDv = bass.AP(tensor=pD.tensor, offset=pD.offset,
                          ap=[pD.ap[0], [1, 128], [128, 3]])
            nc.vector.tensor_reduce(gmin[:, fi % 8, :], pDv, AX.X, ALU.min)
            if fi % 8 == 7:
                gv = bass.AP(tensor=gmin.tensor, offset=gmin[:].offset,
                             ap=[gmin[:].ap[0], [1, 128], [128, 8]])
                mt, first = macc[(rslot, 'dve')]
                if first:
                    nc.vector.tensor_reduce(mt[:], gv, AX.X, ALU.max)
                    macc[(rslot, 'dve')][1] = False
                else:
                    cmax = temps.tile([128, 128], F32, tag="r_cmax")
                    nc.vector.tensor_reduce(cmax[:], gv, AX.X, ALU.max)
                    nc.vector.tensor_tensor(mt[:], mt[:], cmax[:], ALU.max)

    def render_finish(st, out_ch):
        rslot = st['rslot']
        mD = macc[(rslot, 'dve')][0]
        soft = temps.tile([128, 128], F32, tag="r_soft")
        nc.scalar.activation(soft[:], mD[:], AF.Sigmoid)
        nc.sync.dma_start(out_d.ap()[out_ch], soft[:])

    st1 = render_setup(n1, 0)
    st2 = render_setup(n2, 1)

    # ---- decoder with interleaved renders --------------------------------
    with tc.tile_pool(name="psum_d1", bufs=2, space="PSUM") as psum_d1, \
         tc.tile_pool(name="psum_db", bufs=2, space="PSUM") as psum_db, \
         tc.tile_pool(name="psum_pd", bufs=2, space="PSUM") as psum_pd:
        # dw1: out (256, 16, 16); in = up4(4 blk) + f3(2 blk)
        for ocb in range(2):
            ws6 = [wslice() for _ in range(6)]
            ps = psum_d1.tile([128, 256], F32, tag="dps")
            psv = ps.rearrange("p (a b) -> p a b", a=16)
            first = True
            for gi in range(6):
                src = up4_pad[:, gi] if gi < 4 else f3_pad[:, gi - 4]
                for tap in range(25):
                    dy, dx = tap // 5, tap % 5
                    nc.tensor.matmul(psv, ws6[gi][:, tap, :],
                                     src[:, dy:dy + 16, dx:dx + 16],
                                     start=first, stop=(gi == 5 and tap == 24))
                    first = False
            nc.scalar.activation(
                u1_pad[:, ocb, 2:18, 2:18],
                ps[:], AF.Relu, bias=db1[:, ocb:ocb + 1], scale=1.0)
            render_faces(st1, psum_pd, 8 * ocb, 8 * ocb + 8)

        upsample2(u1_pad[:, :, 2:18, 2:18], u1up_pad[:, :, 2:34, 2:34], 128, 2, 16, 16)

        # dw2: out (128, 32, 32); in = u1up(2 blk) + f2(1 blk)
        dw2s = [wslice() for _ in range(3)]
        for c in range(2):
            ps = psum_db.tile([128, 512], F32, tag="bps")
            psv = ps.rearrange("p (a b) -> p a b", a=16)
            first = True
            for bi in range(3):
                src = u1up_pad[:, bi] if bi < 2 else f2_pad
                for tap in range(25):
                    dy, dx = tap // 5, tap % 5
                    nc.tensor.matmul(psv, dw2s[bi][:, tap, :],
                                     src[:, dy + 16 * c:dy + 16 * c + 16, dx:dx + 32],
                                     start=first, stop=(bi == 2 and tap == 24))
                    first = False
            nc.scalar.activation(
                u2_pad[:, 2 + 16 * c:18 + 16 * c, 2:34],
                ps[:], AF.Relu, bias=db2[:], scale=1.0)
            render_faces(st1, psum_pd, 16 + 8 * c, 24 + 8 * c)
        render_finish(st1, 0)

        u2v = u2_pad.rearrange("p (b y) x -> p b y x", b=1)
        u2upv = u2up_pad.rearrange("p (b y) x -> p b y x", b=1)
        upsample2(u2v[:, :, 2:34, 2:34], u2upv[:, :, 2:66, 2:66], 128, 1, 32, 32)

        # dw3: out (128=64x2, 64, 64); in = u2up(1 blk 128) + f1(64 pair-packed)
        dw3a = wslice()
        _lastsl = wslice().rearrange("p a b -> p (a b)")
        dw3bP = _lastsl[:, 0:15 * 128].rearrange("p (a b) -> p a b", a=15)
        dwoP = _lastsl[:, 15 * 128:15 * 128 + 60].rearrange(
            "p (a b) -> p a b", a=15)
        for c in range(8):
            ps = psum_db.tile([128, 512], F32, tag="bps")
            psv = ps.rearrange("p (a b) -> p a b", a=8)
            for tap in range(25):
                dy, dx = tap // 5, tap % 5
                nc.tensor.matmul(psv, dw3a[:, tap, :],
                                 u2up_pad[:, dy + 8 * c:dy + 8 * c + 8, dx:dx + 64],
                                 start=(tap == 0), stop=False)
            for t, (dy_lo, dx) in enumerate(PAIR_TAPS):
                nc.tensor.matmul(psv, dw3bP[:, t, :],
                                 f1_pad[:, dy_lo + 8 * c:dy_lo + 8 * c + 8, dx:dx + 64],
                                 start=False, stop=(t == 14))
            nc.scalar.activation(
                u3_pad[0:64, 2 + 8 * c:10 + 8 * c, 2:66],
                ps[0:64], AF.Relu, bias=db3[0:64], scale=1.0)
            nc.scalar.activation(
                u3_pad[64:128, 1 + 8 * c:9 + 8 * c, 2:66],
                ps[64:128], AF.Relu, bias=db3[64:128], scale=1.0)
            render_faces(st2, psum_pd, 4 * c, 4 * c + 4)
        render_finish(st2, 2)
        if debug:
            nc.sync.dma_start(dbg["dbg_u3"].ap(), u3_pad[:])

    # ---- dwo + disp -------------------------------------------------------
    dwo_scr = dram.tile([4, 64, 64], F32, tag="dwo_scr")
    dwo_f = dwo_scr.rearrange("c y x -> c (y x)")
    with tc.tile_pool(name="psum_o", bufs=2, space="PSUM") as psum_o, \
         tc.tile_pool(name="psum_u", bufs=2, space="PSUM") as psum_u, \
         tc.tile_pool(name="psum_p2", bufs=2, space="PSUM") as psum_p2:
        for c in range(8):
            ps = psum_o.tile([4, 512], F32, tag="ops")
            psv = ps.rearrange("p (a b) -> p a b", a=8)
            for t, (dy_lo, dx) in enumerate(PAIR_TAPS):
                nc.tensor.matmul(psv, dwoP[:, t, :],
                                 u3_pad[:, dy_lo + 8 * c:dy_lo + 8 * c + 8, dx:dx + 64],
                                 start=(t == 0), stop=(t == 14))
            dt_ = chunks.tile([4, 512], F32, tag="dwot")
            nc.scalar.activation(dt_[:], ps[:], AF.Tanh, bias=dbo[:], scale=1.0)
            nc.sync.dma_start(dwo_f[:, ts(c, 512)], dt_[:])

        # disp: repartition [4,64,64] -> [64, 4, 64], upsample-y via matmul,
        # upsample-x via strided vector ops -> disp_sb [128, 4, 128] fp32
        d64 = feat.tile([64, 4, 64], F32, tag="d64")
        src = bass.AP(tensor=dwo_scr.tensor, offset=dwo_scr.offset,
                      ap=[[64, 64], [4096, 4], [1, 64]])
        nc.sync.dma_start(d64[:], src)
        for ch in range(4):
            pu = psum_u.tile([128, 64], F32, tag="ups")
            nc.tensor.matmul(pu[:], U64[:], d64[:, ch, :], start=True, stop=True)
            dch = disp_sb[:, ch, :]
            tb = temps.tile([128, 63], F32, tag="disptb")
            nc.vector.tensor_copy(dch[:, 0:1], pu[:, 0:1])
            nc.vector.tensor_scalar_mul(tb[:], pu[:, 0:63], 1.0 / 3.0)
            nc.vector.tensor_tensor(tb[:], tb[:], pu[:, 1:64], ALU.add)
            nc.vector.tensor_scalar_mul(dch[:, 2:127:2], tb[:], 0.75)
            nc.vector.tensor_scalar_mul(tb[:], pu[:, 1:64], 1.0 / 3.0)
            nc.vector.tensor_tensor(tb[:], tb[:], pu[:, 0:63], ALU.add)
            nc.vector.tensor_scalar_mul(dch[:, 1:126:2], tb[:], 0.75)
            nc.vector.tensor_copy(dch[:, 127:128], pu[:, 63:64])
        if debug:
            nc.sync.dma_start(dbg["dbg_disp"].ap(), disp_sb[:])

        # ---- deformation iterations --------------------------------------
        def sample_prep(nodes_t, tag):
            tp = psum_s.tile([1, 33], F32, tag="sps")
            nc.tensor.transpose(tp[:], nodes_t[:, 1:2], IDENT[0:33, 0:33])
            ypr = small.tile([1, 33], F32, tag=f"ypr{tag}")
            nc.vector.tensor_scalar(ypr[:], tp[:], -64.0, 63.5, ALU.mult, ALU.add)
            pyb = psum_s.tile([128, 33], F32, tag="sps")
            nc.tensor.matmul(pyb[:], ONES[:], ypr[:], start=True, stop=True)
            wy = small.tile([128, 33], F32, tag=f"wy{tag}")
            wyn = small.tile([128, 33], F32, tag=f"wyn{tag}")
            nc.vector.tensor_scalar_sub(wy[:], pyb[:], IOTAY[:])
            nc.vector.tensor_scalar_mul(wyn[:], wy[:], -1.0)
            nc.vector.tensor_tensor(wy[:], wy[:], wyn[:], ALU.max)
            nc.vector.tensor_scalar(wy[:], wy[:], -1.0, 1.0, ALU.mult, ALU.add)
            nc.vector.tensor_scalar_max(wy[:], wy[:], 0.0)
            xc = small.tile([33, 1], F32, tag=f"xc{tag}")
            nc.vector.tensor_scalar(xc[:], nodes_t[:, 0:1], 64.0, 63.5, ALU.mult, ALU.add)
            wx = small.tile([33, 128], F32, tag=f"wx{tag}")
            wxn = small.tile([33, 128], F32, tag=f"wxn{tag}")
            nc.vector.tensor_scalar_sub(wx[:], IOTAX33[:], xc[:])
            nc.vector.tensor_scalar_mul(wxn[:], wx[:], -1.0)
            nc.vector.tensor_tensor(wx[:], wx[:], wxn[:], ALU.max)
            nc.vector.tensor_scalar(wx[:], wx[:], -1.0, 1.0, ALU.mult, ALU.add)
            nc.vector.tensor_scalar_max(wx[:], wx[:], 0.0)
            return wy, wx

        def sample_all(wy, wx, tag):
            pssm = psum_s.tile([33, 512], F32, tag="sps")
            nc.tensor.matmul(pssm[:], wy[:],
                             disp_sb.rearrange("p c x -> p (c x)"),
                             start=True, stop=True)
            prod = temps.tile([33, 4, 128], F32, tag="sp")
            wx_b = bass.AP(tensor=wx.tensor, offset=wx[:].offset,
                           ap=[wx[:].ap[0], [0, 4], [1, 128]])
            nc.vector.tensor_tensor(prod[:], pssm.rearrange("p (c x) -> p c x", c=4),
                                    wx_b, ALU.mult)
            dP = small.tile([33, 4], F32, tag=f"dP{tag}")
            nc.vector.tensor_reduce(dP[:], prod[:], AX.X, ALU.add)
            return dP

        # interleave the two independent deformation chains
        for it in range(ITER):
            wy1, wx1 = sample_prep(n1, "c1")
            wy2, wx2 = sample_prep(n2, "c2")
            dP1 = sample_all(wy1, wx1, "s1")
            dP2 = sample_all(wy2, wx2, "s2")
            n1n = nodes_p.tile([33, 2], F32, tag=f"n1_{it + 1}")
            nc.vector.tensor_tensor(n1n[:, 0:1], n1[:, 0:1], dP1[:, 0:1], ALU.add)
            nc.vector.tensor_tensor(n1n[:, 1:2], n1[:, 1:2], dP1[:, 1:2], ALU.subtract)
            n1 = n1n
            n2n = nodes_p.tile([33, 2], F32, tag=f"n2_{it + 1}")
            t2a = temps.tile([33, 2], F32, tag="t2a")
            t2b = temps.tile([33, 2], F32, tag="t2b")
            nc.vector.tensor_scalar_mul(t2a[:], dP2[:, 2:4], W2M[:])
            nc.vector.tensor_scalar_mul(t2b[:], dP2[:, 0:2], W0M[:])
            nc.vector.tensor_tensor(t2a[:], t2a[:], t2b[:], ALU.add)
            nc.vector.tensor_tensor(n2n[:, 0:1], n2[:, 0:1], t2a[:, 0:1], ALU.add)
            nc.vector.tensor_tensor(n2n[:, 1:2], n2[:, 1:2], t2a[:, 1:2], ALU.subtract)
            n2 = n2n

        if debug:
            nc.sync.dma_start(dbg["dbg_n1"].ap(), n1[:])
            nc.sync.dma_start(dbg["dbg_n2"].ap(), n2[:])

        # final renders, interleaved, with GpSimd offload
        st3 = render_setup(n1, 2)
        st4 = render_setup(n2, 3)
        for k in range(8):
            render_faces(st3, psum_p2, 4 * k, 4 * k + 4, gp_mod=3)
            render_faces(st4, psum_p2, 4 * k, 4 * k + 4, gp_mod=3)
        render_finish(st3, 1)
        render_finish(st4, 3)

    ctx.close()


# ---------------------------------------------------------------------------
# public entry point
# ---------------------------------------------------------------------------

def _get_program(debug=False):
    key = ("prog", debug)
    if key not in _CACHE:
        _CACHE[key] = _build_program(debug)
    return _CACHE[key]


def make_in_maps(host, img):
    in_maps = []
    for c in range(N_CORES):
        m = dict(host)
        m["img"] = img[c, 0]
        in_maps.append(m)
    return in_maps


def kernel(**inputs):
    from concourse import bass_utils

    nc = _get_program(debug=_CACHE.get("debug_mode", False))
    if "host" not in _CACHE:
        _CACHE["host"] = _prep_host(inputs)
    host = _CACHE["host"]

    img = np.asarray(inputs["img"], np.float32)   # (8, 1, 128, 128)
    in_maps = make_in_maps(host, img)
    res = bass_utils.run_bass_kernel_spmd(nc, in_maps, core_ids=list(range(N_CORES)))
    _CACHE["last_results"] = res
    out = np.stack([res.results[c]["out"] for c in range(N_CORES)], 0)
    return out.astype(np.float32)



# revision 18
# speedup vs baseline: 1.0541x; 1.0541x over previous
"""Trainium2 Bass kernel for nn_CardaicCircleNet_78675210928495.

Strategy: pure batch data-parallelism - 8 images, one per NeuronCore.
Per core the full forward pass runs on-chip:
  - convs as shifted matmuls (channels on partitions, spatial free),
    accumulating in PSUM; fp16 operands, fp32 accumulate; 1024-col chunks
  - input normalization computed on-chip; conv1 via flat im2col built
    through a DRAM bounce of the normalized padded image
  - conv1 / dw3 write both the plain and y-shifted (dy-pair) copies of
    their output directly (stationary columns duplicated), removing the
    serial SBUF shift DMAs
  - big conv weights streamed as fine-grained [128,25,128] slices with
    deep prefetch on the scalar DMA queue
  - soft rasterizer: per-edge signed distance is affine in pixel coords ->
    one [2,128]x[2,384] fp32r matmul per face, min/max on DVE with some
    faces offloaded to GpSimd via an ACT fp16 copy; renders interleaved
    with decoder conv chunks so DVE work hides under PE work
  - grid_sample as separable bilinear hat weights -> matmul over rows +
    masked reduce over cols
"""
import os
import sys

for _p in ("/opt/trn_rl_repo", "/root/.axon_site/_ro/trn_rl_repo"):
    if os.path.isdir(_p) and _p not in sys.path:
        sys.path.insert(0, _p)

import numpy as np

IMG = 128
N_FACES = 32
V = 33
CP0 = 16
SHARP = 128.0
ITER = 3
N_CORES = 8

_CACHE = {}

# (name, shape) of consts packed into the fp32 / fp16 blobs, in order
_F32SPEC = [('eb1_2', (128, 1)), ('eb2', (128, 1)), ('eb3', (128, 2)),
            ('eb4', (128, 4)),
            ('cb1', (128, 2)), ('cb2', (128, 1)), ('lb1', (1, 400)),
            ('lb2', (1, 200)), ('lb3', (1, 6)), ('db1', (128, 2)),
            ('db2', (128, 1)), ('db3_2', (128, 1)), ('dbo', (4, 1)),
            ('cst_xs128', (128, 128)), ('cst_ly', (2, 128)),
            ('cst_niotay2', (128, 1)), ('cst_iotax33', (33, 128)),
            ('cst_onecol', (128, 1)), ('cst_sumw1', (128, 1)),
            ('cst_nodes1', (33, 2)), ('cst_nodes2', (33, 2)),
            ('cst_g0', (33, 96)), ('cst_g1', (33, 96)),
            ('cst_w2m', (33, 1)), ('cst_w0m', (33, 1))]
_F16SPEC = [('w2P', (128, 15, 128)), ('cw2T', (128, 2, 128)),
            ('lw2T', (100, 4, 200)), ('lw3T', (100, 2, 6)),
            ('cst_u64f', (64, 128))]

# weight stream: slice s is [128, 25, 128] fp16; order of consumption
# w3(2) w4(4x2) cw1(2x4) lw1(2) dw1(2x6) dw2(3) dw3a(1) dw3b+dwo(1)
N_STREAM = 2 + 8 + 8 + 2 + 12 + 3 + 1 + 1


def _blob_offsets(spec):
    off = {}
    c = 0
    for nm, sh in spec:
        w = 1
        for s in sh[1:]:
            w *= s
        off[nm] = (c, sh)
        c += w
    return off, c


# ---------------------------------------------------------------------------
# host-side constant / weight preparation (layout only, cached)
# ---------------------------------------------------------------------------

def _circles_np():
    th = 2.0 * np.pi * np.arange(N_FACES) / N_FACES
    ring = np.stack([np.cos(th), np.sin(th)], 1)
    nodes1 = np.vstack([0.5 * ring, [[0.0, 0.0]]]).astype(np.float32)
    nodes2 = np.vstack([0.3 * ring + [0.1, 0.0], [[0.1, 0.0]]]).astype(np.float32)
    faces = np.stack([np.arange(N_FACES), (np.arange(N_FACES) + 1) % N_FACES,
                      np.full(N_FACES, N_FACES)], 1)
    return nodes1, nodes2, faces


def _conv_wT(w, icb_count, ocb, oc_per_blk=128):
    """w: (OC, IC, 5, 5) -> [128, icb_count, 25, oc_per_blk] fp16 for ocb slice."""
    OC, IC = w.shape[:2]
    out = np.zeros((128, icb_count, 25, oc_per_blk), np.float16)
    for icb in range(icb_count):
        ic0 = icb * 128
        icn = min(128, IC - ic0)
        blk = w[ocb * oc_per_blk:(ocb + 1) * oc_per_blk, ic0:ic0 + icn]
        out[:icn, icb] = blk.transpose(1, 2, 3, 0).reshape(icn, 25, -1).astype(np.float16)
    return out


def _upmat64():
    """U[iny=64, outy=128] fp32: bilinear x2 upsample with edge clamp (lhsT)."""
    U = np.zeros((64, 128), np.float32)
    for j in range(64):
        jm = max(j - 1, 0)
        jp = min(j + 1, 63)
        U[jm, 2 * j] += 0.25
        U[j, 2 * j] += 0.75
        U[j, 2 * j + 1] += 0.75
        U[jp, 2 * j + 1] += 0.25
    return U


def _pair_pack(wT64, oc):
    """wT64: [64, 25, oc] -> [128, 15, oc]: taps (dy_lo in 0,2,4) x dx;
    rows 64-127 = dy_lo+1 tap (zero when dy_lo==4)."""
    out = np.zeros((128, 15, oc), np.float16)
    t = 0
    for dy_lo in (0, 2, 4):
        for dx in range(5):
            out[0:64, t] = wT64[:, dy_lo * 5 + dx]
            if dy_lo + 1 <= 4:
                out[64:128, t] = wT64[:, (dy_lo + 1) * 5 + dx]
            t += 1
    return out


def _prep_host(inputs):
    p = {k: np.asarray(v) for k, v in inputs.items()}
    d = {}
    # conv1 stationary: [25 taps, 128] = oc duplicated twice (plain+shifted)
    w1T = p['ew1'][:, 0].transpose(1, 2, 0).reshape(25, 64)
    d['w1P'] = np.concatenate([w1T, w1T], axis=1).astype(np.float16)  # (25,128)

    d['w2P'] = _pair_pack(_conv_wT(p['ew2'], 1, 0)[:64, 0], 128)   # [128, 15, 128]
    cw2 = p['cw2'][:, :, 0, 0]                              # (128, 256)
    d['cw2T'] = np.stack([cw2[:, k * 128:(k + 1) * 128].T for k in range(2)], 1).astype(np.float16)
    d['lw2T'] = p['lw2'].reshape(4, 100, 200).transpose(1, 0, 2).astype(np.float16)
    d['lw3T'] = p['lw3'].reshape(2, 100, 6).transpose(1, 0, 2).astype(np.float16)
    # dw3: duplicate output columns (plain + y-shifted copies of u3)
    dw3 = _conv_wT(p['dw3'], 2, 0, 64)                      # [128, 2, 25, 64]
    dw3a2 = np.concatenate([dw3[:, 0], dw3[:, 0]], axis=-1)               # [128,25,128]
    b = _pair_pack(dw3[:64, 1], 64)                                       # [128,15,64]
    dw3bP2 = np.concatenate([b, b], axis=-1)                              # [128,15,128]
    dwoP = _pair_pack(_conv_wT(p['dwo'], 1, 0, 4)[:64, 0], 4)             # [128, 15, 4]

    # ---- weight stream slices [128, S, 25*128] fp16, consumption order ----
    SL = 25 * 128
    slices = []
    for ocb in range(2):
        slices.append(_conv_wT(p['ew3'], 1, ocb)[:, 0].reshape(128, SL))
    for ocb in range(4):
        w4 = _conv_wT(p['ew4'], 2, ocb)                          # [128,2,25,128]
        slices.append(w4[:, 0].reshape(128, SL))
        slices.append(w4[:, 1].reshape(128, SL))
    for ocb in range(2):
        cw1 = _conv_wT(p['cw1'], 4, ocb)                         # [128,4,25,128]
        for icb in range(4):
            slices.append(cw1[:, icb].reshape(128, SL))
    lw1 = p['lw1'].reshape(128, 16, 400).astype(np.float16)
    slices.append(lw1[:, 0:8].reshape(128, SL))
    slices.append(lw1[:, 8:16].reshape(128, SL))
    for ocb in range(2):
        dw1 = _conv_wT(p['dw1'], 6, ocb)                         # [128,6,25,128]
        for ich in range(6):
            slices.append(dw1[:, ich].reshape(128, SL))
    dw2 = _conv_wT(p['dw2'], 3, 0)                               # [128,3,25,128]
    for icb in range(3):
        slices.append(dw2[:, icb].reshape(128, SL))
    slices.append(dw3a2.reshape(128, SL))
    last = np.zeros((128, SL), np.float16)
    last[:, :15 * 128] = dw3bP2.reshape(128, 15 * 128)
    last[:, 15 * 128:15 * 128 + 60] = dwoP.reshape(128, 60)
    slices.append(last)
    assert len(slices) == N_STREAM
    d['wstream'] = np.stack(slices, 1)

    # biases fp32
    eb1 = p['eb1'].reshape(64, 1).astype(np.float32)
    d['eb1_2'] = np.concatenate([eb1, eb1], axis=0)                   # [128,1]
    d['eb2'] = p['eb2'].reshape(128, 1).astype(np.float32)
    d['eb3'] = p['eb3'].reshape(2, 128).T.copy().astype(np.float32)   # [128, 2]
    d['eb4'] = p['eb4'].reshape(4, 128).T.copy().astype(np.float32)   # [128, 4]
    d['cb1'] = p['cb1'].reshape(2, 128).T.copy().astype(np.float32)
    d['cb2'] = p['cb2'].reshape(128, 1).astype(np.float32)
    d['lb1'] = p['lb1'].reshape(1, 400).astype(np.float32)
    d['lb2'] = p['lb2'].reshape(1, 200).astype(np.float32)
    d['lb3'] = p['lb3'].reshape(1, 6).astype(np.float32)
    d['db1'] = p['db1'].reshape(2, 128).T.copy().astype(np.float32)
    d['db2'] = p['db2'].reshape(128, 1).astype(np.float32)
    db3 = p['db3'].reshape(64, 1).astype(np.float32)
    d['db3_2'] = np.concatenate([db3, db3], axis=0)                   # [128,1]
    d['dbo'] = p['dbo'].reshape(4, 1).astype(np.float32)
    # constants
    xs = ((np.arange(IMG) + 0.5) * (2.0 / IMG) - 1.0).astype(np.float32)
    ys = (1.0 - (np.arange(IMG) + 0.5) * (2.0 / IMG)).astype(np.float32)
    d['cst_xs128'] = np.broadcast_to(xs, (128, 128)).copy()
    d['cst_ly'] = np.stack([np.ones(128, np.float32), ys], 0)         # [2, 128]
    d['cst_ones'] = np.ones((1, 128), np.float32)
    d['cst_niotay2'] = (63.5 - np.arange(128, dtype=np.float32)).reshape(128, 1)
    d['cst_iotax33'] = np.broadcast_to(np.arange(128, dtype=np.float32), (33, 128)).copy()
    d['cst_onecol'] = np.ones((128, 1), np.float32)
    # per-oc sum of conv1 weights (duplicated for the dy-pair copy)
    sumw1 = p['ew1'].sum(axis=(1, 2, 3)).reshape(64, 1).astype(np.float32)
    d['cst_sumw1'] = np.concatenate([sumw1, sumw1], axis=0)           # [128,1]
    d['cst_negones2'] = np.full((2, 1), -1.0, np.float32)
    d['cst_u64f'] = _upmat64().astype(np.float16)
    nodes1, nodes2, faces = _circles_np()
    d['cst_nodes1'] = nodes1
    d['cst_nodes2'] = nodes2
    G0 = np.zeros((33, 96), np.float32)
    G1 = np.zeros((33, 96), np.float32)
    nxt = np.roll(np.arange(3), -1)
    for f in range(N_FACES):
        for j in range(3):
            G0[faces[f][j], f * 3 + j] = 1.0
            G1[faces[f][nxt[j]], f * 3 + j] = 1.0
    d['cst_g0'] = G0
    d['cst_g1'] = G1
    idx = np.arange(V)
    d['cst_w2m'] = (idx <= CP0).astype(np.float32).reshape(33, 1)
    d['cst_w0m'] = ((idx >= CP0).astype(np.float32)
                    + (idx == V - 1).astype(np.float32)).reshape(33, 1)

    off32, w32 = _blob_offsets(_F32SPEC)
    blob32 = np.zeros((128, w32), np.float32)
    for nm, sh in _F32SPEC:
        a = d[nm]
        c0, _ = off32[nm]
        blob32[:a.shape[0], c0:c0 + int(np.prod(sh[1:]))] = a.reshape(a.shape[0], -1)
        del d[nm]
    off16, w16 = _blob_offsets(_F16SPEC)
    blob16 = np.zeros((128, w16), np.float16)
    for nm, sh in _F16SPEC:
        a = d[nm]
        c0, _ = off16[nm]
        blob16[:a.shape[0], c0:c0 + int(np.prod(sh[1:]))] = a.reshape(a.shape[0], -1)
        del d[nm]
    d['blob32'] = blob32
    d['blob16'] = blob16
    return d


# ---------------------------------------------------------------------------
# device program
# ---------------------------------------------------------------------------

def _build_program(debug=False):
    import concourse.bass as bass
    import concourse.tile as tile
    from concourse import mybir, bacc
    from concourse.masks import make_identity

    F32 = mybir.dt.float32
    F16 = mybir.dt.float16

    nc = bacc.Bacc("TRN2", num_devices=N_CORES, debug=False)

    din = {}
    def dt_in(name, shape, dtype=F32):
        din[name] = nc.dram_tensor(name, list(shape), dtype, kind="ExternalInput")
        return din[name]

    dt_in("img", (128, 128))
    dt_in("w1P", (25, 128), F16)
    dt_in("wstream", (128, N_STREAM, 25 * 128), F16)
    _o32, _w32 = _blob_offsets(_F32SPEC)
    _o16, _w16 = _blob_offsets(_F16SPEC)
    dt_in("blob32", (128, _w32))
    dt_in("blob16", (128, _w16), F16)
    for nm, sh in [("cst_ones", (1, 128)), ("cst_negones2", (2, 1))]:
        dt_in(nm, sh)

    out_d = nc.dram_tensor("out", [4, 128, 128], F32, kind="ExternalOutput")
    dbg = {}
    if debug:
        for nm, sh, dt_ in [("dbg_f1", (128, 68, 68), F16),
                            ("dbg_f4", (128, 4, 12, 12), F16),
                            ("dbg_cb", (128, 16), F16), ("dbg_aff", (1, 6), F32),
                            ("dbg_u3", (128, 68, 68), F16),
                            ("dbg_disp", (128, 4, 128), F16),
                            ("dbg_n1", (33, 2), F32), ("dbg_n2", (33, 2), F32)]:
            dbg[nm] = nc.dram_tensor(nm, list(sh), dt_, kind="ExternalOutput")

    with tile.TileContext(nc) as tc:
        _emit(nc, tc, tile, bass, mybir, din, out_d, dbg, make_identity, debug)

    nc.compile()
    return nc


def _emit(nc, tc, tile, bass, mybir, din, out_d, dbg, make_identity, debug):
    F32 = mybir.dt.float32
    F32R = mybir.dt.float32r
    F16 = mybir.dt.float16
    AF = mybir.ActivationFunctionType
    ALU = mybir.AluOpType
    AX = mybir.AxisListType
    ts = bass.ts

    from contextlib import ExitStack
    ctx = ExitStack()

    consts = ctx.enter_context(tc.tile_pool(name="consts", bufs=1))
    feat = ctx.enter_context(tc.tile_pool(name="feat", bufs=1))
    chunks = ctx.enter_context(tc.tile_pool(name="chunks", bufs=3))
    temps = ctx.enter_context(tc.tile_pool(name="temps", bufs=2))
    small = ctx.enter_context(tc.tile_pool(name="small", bufs=2))
    nodes_p = ctx.enter_context(tc.tile_pool(name="nodes", bufs=10))
    psum_s = ctx.enter_context(tc.tile_pool(name="psum_s", bufs=2, space="PSUM"))
    dram = ctx.enter_context(tc.tile_pool(name="dram", bufs=1, space="DRAM"))
    rendp = ctx.enter_context(tc.tile_pool(name="rendp", bufs=2))
    grpp = ctx.enter_context(tc.tile_pool(name="grpp", bufs=2))

    _o32, _ = _blob_offsets(_F32SPEC)
    _o16, _ = _blob_offsets(_F16SPEC)

    # ---- earliest DMAs ----------------------------------------------------
    t_img = small.tile([128, 128], F32, tag="timg")
    nc.sync.dma_start(t_img[:], din["img"].ap())
    w1P = consts.tile([25, 128], F16, tag="w1P")
    nc.sync.dma_start(w1P[:], din["w1P"].ap())
    NEG2 = consts.tile([2, 1], F32, tag="neg2")
    nc.sync.dma_start(NEG2[:], din["cst_negones2"].ap())
    ONES = consts.tile([1, 128], F32, tag="ones")
    nc.sync.dma_start(ONES[:], din["cst_ones"].ap())
    B32 = consts.tile([128, _blob_offsets(_F32SPEC)[1]], F32, tag="b32")
    nc.scalar.dma_start(B32[:], din["blob32"].ap())
    B16 = consts.tile([128, _blob_offsets(_F16SPEC)[1]], F16, tag="b16")
    nc.scalar.dma_start(B16[:], din["blob16"].ap())

    def c32(nm):
        c0, sh = _o32[nm]
        w = 1
        for s in sh[1:]:
            w *= s
        ap = B32[0:sh[0], c0:c0 + w]
        if len(sh) == 3:
            ap = ap.rearrange("p (a b) -> p a b", a=sh[1])
        return ap

    def c16(nm):
        c0, sh = _o16[nm]
        w = 1
        for s in sh[1:]:
            w *= s
        ap = B16[0:sh[0], c0:c0 + w]
        if len(sh) == 3:
            ap = ap.rearrange("p (a b) -> p a b", a=sh[1])
        return ap

    w2P = c16("w2P"); cw2T = c16("cw2T"); lw2T = c16("lw2T")
    lw3T = c16("lw3T"); U64F = c16("cst_u64f")
    eb1 = c32("eb1_2"); eb2 = c32("eb2"); eb3 = c32("eb3"); eb4 = c32("eb4")
    cb1 = c32("cb1"); cb2 = c32("cb2")
    lb1 = c32("lb1"); lb2 = c32("lb2"); lb3 = c32("lb3")
    db1 = c32("db1"); db2 = c32("db2"); db3 = c32("db3_2"); dbo = c32("dbo")
    XS = c32("cst_xs128"); LY = c32("cst_ly"); NIOTAY2 = c32("cst_niotay2")
    IOTAX33 = c32("cst_iotax33"); ONECOL = c32("cst_onecol")
    SUMW1 = c32("cst_sumw1")
    NODES1 = c32("cst_nodes1"); NODES2 = c32("cst_nodes2")
    G0 = c32("cst_g0"); G1 = c32("cst_g1")
    W2M = c32("cst_w2m"); W0M = c32("cst_w0m")
    IDENT = consts.tile([128, 128], F32, tag="ident")
    make_identity(nc, IDENT)

    # ---- PE warm-up: dense dummy matmuls so HAM unthrottles before conv1 ---
    wmt = consts.tile([128, 512], F16, tag="warm")
    nc.vector.memset(wmt[:], 0.0)
    with tc.tile_pool(name="psum_w", bufs=1, space="PSUM") as psum_w:
        wps = psum_w.tile([128, 512], F32, tag="wps")
        for _ in range(16):
            nc.tensor.matmul(wps[:], wmt[:, 0:128], wmt[:],
                             start=True, stop=True)

    # ---- persistent feature buffers (border-only zeroing) -----------------
    f1_pad = feat.tile([128, 68, 68], F16, tag="f1_pad")
    f2_pad = feat.tile([128, 36, 36], F16, tag="f2_pad")
    f3_pad = feat.tile([128, 2, 20, 20], F16, tag="f3_pad")
    f4_pad = feat.tile([128, 4, 12, 12], F16, tag="f4_pad")
    up4_pad = feat.tile([128, 4, 20, 20], F16, tag="up4_pad")
    u1_pad = feat.tile([128, 2, 20, 20], F16, tag="u1_pad")
    u1up_pad = feat.tile([128, 2, 36, 36], F16, tag="u1up_pad")
    u2_pad = feat.tile([128, 36, 36], F16, tag="u2_pad")
    u2up_pad = feat.tile([128, 68, 68], F16, tag="u2up_pad")
    u3_pad = feat.tile([128, 68, 68], F16, tag="u3_pad")
    disp_sb = feat.tile([128, 4, 128], F16, tag="disp")

    def zero_borders(t, nblk, H, eng):
        # zero the 2-px border ring of each [128, H, H] block
        if nblk == 1:
            v = t.rearrange("p (b y) x -> p b y x", b=1)
        else:
            v = t
        eng.memset(v[:, :, 0:2, :], 0.0)
        eng.memset(v[:, :, H - 2:H, :], 0.0)
        eng.memset(v[:, :, 2:H - 2, 0:2], 0.0)
        eng.memset(v[:, :, 2:H - 2, H - 2:H], 0.0)

    zero_borders(f1_pad, 1, 68, nc.gpsimd)
    zero_borders(f2_pad, 1, 36, nc.gpsimd)
    zero_borders(f3_pad, 2, 20, nc.gpsimd)
    zero_borders(f4_pad, 4, 12, nc.gpsimd)
    zero_borders(up4_pad, 4, 20, nc.gpsimd)
    zero_borders(u1_pad, 2, 20, nc.gpsimd)
    zero_borders(u1up_pad, 2, 36, nc.gpsimd)
    zero_borders(u2_pad, 1, 36, nc.gpsimd)
    zero_borders(u2up_pad, 1, 68, nc.gpsimd)
    zero_borders(u3_pad, 1, 68, nc.gpsimd)
    # f1/u3 shifted-copy rows outside interior writes (see conv1/dw3)
    nc.gpsimd.memset(f1_pad[64:128, 65:68, :], 0.0)
    nc.gpsimd.memset(u3_pad[64:128, 65:68, :], 0.0)

    # ---- stage 0: min/max -> scale/shift; conv1 runs on the RAW image -----
    # (normalization folded into conv1's relu epilogue: scale=sc per-pixel is
    #  uniform, bias = -mn*sc*sum(w)+b; padding uses value mn == norm-zero)
    r2 = small.tile([128, 2], F32, tag="r2")
    nc.vector.tensor_reduce(r2[:, 0:1], t_img[:], AX.X, ALU.min)
    nc.vector.tensor_reduce(r2[:, 1:2], t_img[:], AX.X, ALU.max, negate=True)
    tr2 = psum_s.tile([2, 128], F32, tag="sps")
    nc.tensor.transpose(tr2[:], r2[:], IDENT[:])
    rmm = small.tile([2, 1], F32, tag="rmm")
    nc.vector.tensor_reduce(rmm[:], tr2[:], AX.X, ALU.min)   # [mn, -mx]
    pmn = psum_s.tile([128, 1], F32, tag="sps")
    nc.tensor.matmul(pmn[:], ONES[0:1, :], rmm[0:1, 0:1], start=True, stop=True)
    mn_col = small.tile([128, 1], F32, tag="mncol")
    nc.scalar.copy(mn_col[:], pmn[:])
    # raw padded image (fp16): interior independent of the min/max chain
    nimg = small.tile([128, 132], F16, tag="nimg")
    nc.vector.tensor_copy(nimg[:, 2:130], t_img[:])
    nc.vector.tensor_scalar(nimg[:, 0:2], t_img[:, 0:2], 0.0, mn_col[:],
                            ALU.mult, ALU.add)
    nc.vector.tensor_scalar(nimg[:, 130:132], t_img[:, 0:2], 0.0, mn_col[:],
                            ALU.mult, ALU.add)
    zrow = small.tile([2, 132], F16, tag="zrow")
    nc.vector.tensor_scalar(zrow[:], B32[0:2, 0:132], 0.0, mn_col[0:2, :],
                            ALU.mult, ALU.add)
    pad_scr = dram.tile([132, 132], F16, tag="pad_scr")
    nc.sync.dma_start(pad_scr[0:2], zrow[:])
    nc.sync.dma_start(pad_scr[130:132], zrow[:])
    nc.sync.dma_start(pad_scr[2:38], nimg[0:36, :])
    nc.sync.dma_start(pad_scr[38:70], nimg[36:68, :])
    nc.sync.dma_start(pad_scr[70:102], nimg[68:100, :])
    nc.sync.dma_start(pad_scr[102:130], nimg[100:128, :])
    # rest of the scale/shift chain (only needed by conv1's epilogue)
    pden = psum_s.tile([1, 1], F32, tag="sps")
    nc.tensor.matmul(pden[:], NEG2[:], rmm[:], start=True, stop=True)  # mx-mn
    den = small.tile([1, 1], F32, tag="den")
    nc.vector.tensor_scalar_add(den[:], pden[:], 0.01)
    sc = small.tile([1, 1], F32, tag="sc")
    nc.vector.reciprocal(sc[:], den[:])
    shp = small.tile([1, 1], F32, tag="shp")
    nc.vector.tensor_tensor(shp[:], rmm[0:1, :], sc[:], ALU.mult)   # mn*sc
    scsh = small.tile([1, 2], F32, tag="scsh")
    nc.vector.tensor_copy(scsh[0:1, 0:1], sc[:])
    nc.vector.tensor_scalar_mul(scsh[0:1, 1:2], shp[:], -1.0)
    pbc = psum_s.tile([128, 2], F32, tag="sps")
    nc.tensor.matmul(pbc[:], ONES[0:1, :], scsh[:], start=True, stop=True)
    bc = small.tile([128, 2], F32, tag="bc")                  # [sc, -mn*sc]
    nc.scalar.copy(bc[:], pbc[:])
    bias1 = small.tile([128, 1], F32, tag="bias1")
    nc.vector.tensor_scalar(bias1[:], SUMW1[:], bc[:, 1:2], eb1[:],
                            ALU.mult, ALU.add)

    PAIR_TAPS = [(dy_lo, dx) for dy_lo in (0, 2, 4) for dx in range(5)]

    def relu_pool(ps, oc, nrows, W_out, bias_ap, dst_ap):
        """relu(ps+bias) -> fp16 -> 2x2 maxpool -> dst_ap [oc, nrows/2, W_out/2]."""
        ct = chunks.tile([oc, nrows, W_out], F16, tag="ct")
        nc.scalar.activation(ct.rearrange("p a b -> p (a b)"), ps,
                             AF.Relu, bias=bias_ap, scale=1.0)
        mr = temps.tile([oc, nrows // 2, W_out], F16, tag="mr")
        nc.vector.tensor_tensor(mr[:], ct[:, 0::2, :], ct[:, 1::2, :], ALU.max)
        nc.vector.tensor_tensor(dst_ap, mr[:, :, 0::2], mr[:, :, 1::2], ALU.max)

    # ---- weight stream ----------------------------------------------------
    upt = ctx.enter_context(tc.tile_pool(name="upt", bufs=1))
    wpool = ctx.enter_context(tc.tile_pool(name="wpool", bufs=5))
    _snext = [0]

    def wslice():
        s = _snext[0]
        _snext[0] += 1
        t = wpool.tile([128, 25, 128], F16, tag="ws")
        nc.scalar.dma_start(t.rearrange("p a b -> p (a b)"),
                            din["wstream"].ap()[:, s])
        return t

    # ---- conv1 + conv2 + conv3 (shared psum pool, 1024-col chunks) --------
    with tc.tile_pool(name="i2c", bufs=2) as i2cp, \
         tc.tile_pool(name="psum_c", bufs=3, space="PSUM") as psum_c:
        imgp = pad_scr[:].rearrange("a b -> (a b)")
        I2Cv = None
        for c in range(32):
            if c % 8 == 0:
                # 4 pipelined slabs; consumed flat range per partition is
                # [0, 31*132+128) at row offset 32k
                I2C = i2cp.tile([25, 32 * 132], F16, tag="i2c", name="I2C")
                slab_src = bass.AP(tensor=imgp.tensor,
                                   offset=imgp.offset + (32 * 132) * (c // 8),
                                   ap=[[132, 5], [1, 5], [1, 31 * 132 + 128]])
                nc.sync.dma_start(I2C[:, 0:31 * 132 + 128], slab_src)
                I2Cv = I2C.rearrange("p (y x) -> p y x", x=132)
            cc = c % 8
            ps = psum_c.tile([128, 512], F32, tag="cps")
            nc.tensor.matmul(ps.rearrange("p (a b) -> p a b", a=4),
                             w1P[:], I2Cv[:, 4 * cc:4 * cc + 4, 0:128],
                             start=True, stop=True)
            ct = chunks.tile([128, 4, 128], F16, tag="ct")
            nc.scalar.activation(ct.rearrange("p a b -> p (a b)"), ps[:],
                                 AF.Relu, bias=bias1[:], scale=bc[:, 0:1])
            mr = temps.tile([128, 2, 128], F16, tag="mr")
            nc.vector.tensor_tensor(mr[:], ct[:, 0::2, :], ct[:, 1::2, :], ALU.max)
            nc.vector.tensor_tensor(f1_pad[0:64, 2 + 2 * c:4 + 2 * c, 2:66],
                                    mr[0:64, :, 0::2], mr[0:64, :, 1::2], ALU.max)
            nc.vector.tensor_tensor(f1_pad[64:128, 1 + 2 * c:3 + 2 * c, 2:66],
                                    mr[64:128, :, 0::2], mr[64:128, :, 1::2], ALU.max)

        if debug:
            nc.sync.dma_start(dbg["dbg_f1"].ap(), f1_pad[:])

        # conv2: dy-pair packed, 8 chunks of 8 out rows
        for c in range(8):
            ps = psum_c.tile([128, 512], F32, tag="cps")
            psv = ps.rearrange("p (a b) -> p a b", a=8)
            for t, (dy_lo, dx) in enumerate(PAIR_TAPS):
                nc.tensor.matmul(psv, w2P[:, t, :],
                                 f1_pad[:, dy_lo + 8 * c:dy_lo + 8 * c + 8, dx:dx + 64],
                                 start=(t == 0), stop=(t == 14))
            relu_pool(ps[:], 128, 8, 64, eb2[:], f2_pad[:, 2 + 4 * c:6 + 4 * c, 2:34])

        # conv3: 2 chunks of 16 out rows per ocb
        w3s = [wslice(), wslice()]
        for c in range(2):
            for ocb in range(2):
                ps = psum_c.tile([128, 512], F32, tag="cps")
                psv = ps.rearrange("p (a b) -> p a b", a=16)
                for tap in range(25):
                    dy, dx = tap // 5, tap % 5
                    nc.tensor.matmul(psv, w3s[ocb][:, tap, :],
                                     f2_pad[:, dy + 16 * c:dy + 16 * c + 16, dx:dx + 32],
                                     start=(tap == 0), stop=(tap == 24))
                relu_pool(ps[:], 128, 16, 32, eb3[:, ocb:ocb + 1],
                          f3_pad[:, ocb, 2 + 8 * c:10 + 8 * c, 2:18])

    # ---- conv4 + cw1 + cw2 + FC ------------------------------------------
    def upsample2(src, dst_interior, P, nblk, H, W):
        up_t = upt.tile([P, nblk, 2 * H, W], F16, tag=f"up_t{H}")
        ta = upt.tile([P, nblk, H - 1, W], F16, tag=f"up_a{H}")
        nc.vector.tensor_copy(up_t[:, :, 0:1, :], src[:, :, 0:1, :])
        nc.vector.tensor_scalar_mul(ta[:], src[:, :, 0:H - 1, :], 1.0 / 3.0)
        nc.vector.tensor_tensor(ta[:], ta[:], src[:, :, 1:H, :], ALU.add)
        nc.vector.tensor_scalar_mul(up_t[:, :, 2:2 * H - 1:2, :], ta[:], 0.75)
        nc.vector.tensor_scalar_mul(ta[:], src[:, :, 1:H, :], 1.0 / 3.0)
        nc.vector.tensor_tensor(ta[:], ta[:], src[:, :, 0:H - 1, :], ALU.add)
        nc.vector.tensor_scalar_mul(up_t[:, :, 1:2 * H - 2:2, :], ta[:], 0.75)
        nc.vector.tensor_copy(up_t[:, :, 2 * H - 1:2 * H, :], src[:, :, H - 1:H, :])
        tb = upt.tile([P, nblk, 2 * H, W - 1], F16, tag=f"up_b{H}")
        nc.vector.tensor_copy(dst_interior[:, :, :, 0:1], up_t[:, :, :, 0:1])
        nc.vector.tensor_scalar_mul(tb[:], up_t[:, :, :, 0:W - 1], 1.0 / 3.0)
        nc.vector.tensor_tensor(tb[:], tb[:], up_t[:, :, :, 1:W], ALU.add)
        nc.vector.tensor_scalar_mul(dst_interior[:, :, :, 2:2 * W - 1:2], tb[:], 0.75)
        nc.vector.tensor_scalar_mul(tb[:], up_t[:, :, :, 1:W], 1.0 / 3.0)
        nc.vector.tensor_tensor(tb[:], tb[:], up_t[:, :, :, 0:W - 1], ALU.add)
        nc.vector.tensor_scalar_mul(dst_interior[:, :, :, 1:2 * W - 2:2], tb[:], 0.75)
        nc.vector.tensor_copy(dst_interior[:, :, :, 2 * W - 1:2 * W],
                              up_t[:, :, :, W - 1:W])

    with tc.tile_pool(name="psum_m", bufs=3, space="PSUM") as psum_m:
        # conv4
        for ocb in range(4):
            wa, wb = wslice(), wslice()
            ps = psum_m.tile([128, 256], F32, tag="mps")
            psv = ps.rearrange("p (a b) -> p a b", a=16)
            first = True
            for bi, w in enumerate((wa, wb)):
                for tap in range(25):
                    dy, dx = tap // 5, tap % 5
                    nc.tensor.matmul(psv, w[:, tap, :],
                                     f3_pad[:, bi, dy:dy + 16, dx:dx + 16],
                                     start=first, stop=(bi == 1 and tap == 24))
                    first = False
            relu_pool(ps[:], 128, 16, 16, eb4[:, ocb:ocb + 1],
                      f4_pad[:, ocb, 2:10, 2:10])
        if debug:
            nc.sync.dma_start(dbg["dbg_f4"].ap(), f4_pad[:])

        # up4 upsample early (DVE) so dw1 can start right after FC
        upsample2(f4_pad[:, :, 2:10, 2:10], up4_pad[:, :, 2:18, 2:18], 128, 4, 8, 8)

        # cw1
        ca = feat.tile([128, 2, 4, 4], F16, tag="ca")
        for ocb in range(2):
            ws4 = [wslice() for _ in range(4)]
            ps_full = psum_m.tile([128, 256], F32, tag="mps")
            ps = ps_full[:, 0:64]
            psv = ps.rearrange("p (a b) -> p a b", a=8)
            first = True
            for bi in range(4):
                for tap in range(25):
                    dy, dx = tap // 5, tap % 5
                    nc.tensor.matmul(psv, ws4[bi][:, tap, :],
                                     f4_pad[:, bi, dy:dy + 8, dx:dx + 8],
                                     start=first, stop=(bi == 3 and tap == 24))
                    first = False
            relu_pool(ps[:], 128, 8, 8, cb1[:, ocb:ocb + 1], ca[:, ocb])

        # cw2 1x1
        ps6 = psum_s.tile([128, 16], F32, tag="sps")
        caf = ca.rearrange("p b y x -> p b (y x)")
        for icb in range(2):
            nc.tensor.matmul(ps6[:], cw2T[:, icb, :], caf[:, icb, :],
                             start=(icb == 0), stop=(icb == 1))
        cbt = feat.tile([128, 16], F16, tag="cb")
        nc.scalar.activation(cbt[:], ps6[:], AF.Relu, bias=cb2[:], scale=1.0)
        if debug:
            nc.sync.dma_start(dbg["dbg_cb"].ap(), cbt[:])

        # FC head (lw1 arrives as two stream slices of 8x400)
        lw1a = wslice().rearrange("p a b -> p (a b)").rearrange(
            "p (a b) -> p a b", a=8)
        lw1b = wslice().rearrange("p a b -> p (a b)").rearrange(
            "p (a b) -> p a b", a=8)
        ps7 = psum_s.tile([1, 400], F32, tag="sps")
        for s in range(16):
            lw1s = lw1a if s < 8 else lw1b
            nc.tensor.matmul(ps7[:], cbt[:, s:s + 1], lw1s[:, s % 8, :],
                             start=(s == 0), stop=(s == 15))
        a1r = small.tile([1, 400], F32, tag="a1r")
        nc.vector.tensor_tensor(a1r[:], ps7[:], lb1[:], ALU.add)
        nc.vector.tensor_scalar_max(a1r[:], a1r[:], 0.0)
        a1c = small.tile([100, 4], F16, tag="a1c")
        for k in range(4):
            pt = psum_s.tile([100, 1], F32, tag="sps")
            nc.tensor.transpose(pt[:], a1r[0:1, ts(k, 100)], IDENT[0:1, 0:1])
            nc.scalar.copy(a1c[:, k:k + 1], pt[:])
        ps8 = psum_s.tile([1, 200], F32, tag="sps")
        for k in range(4):
            nc.tensor.matmul(ps8[:], a1c[:, k:k + 1], lw2T[:, k, :],
                             start=(k == 0), stop=(k == 3))
        a2r = small.tile([1, 200], F32, tag="a2r")
        nc.vector.tensor_tensor(a2r[:], ps8[:], lb2[:], ALU.add)
        nc.vector.tensor_scalar_max(a2r[:], a2r[:], 0.0)
        a2c = small.tile([100, 2], F16, tag="a2c")
        for k in range(2):
            pt = psum_s.tile([100, 1], F32, tag="sps")
            nc.tensor.transpose(pt[:], a2r[0:1, ts(k, 100)], IDENT[0:1, 0:1])
            nc.scalar.copy(a2c[:, k:k + 1], pt[:])
        ps9 = psum_s.tile([1, 6], F32, tag="sps")
        for k in range(2):
            nc.tensor.matmul(ps9[:], a2c[:, k:k + 1], lw3T[:, k, :],
                             start=(k == 0), stop=(k == 1))
        afz = small.tile([1, 6], F32, tag="afz")
        nc.vector.tensor_tensor(afz[:], ps9[:], lb3[:], ALU.add)
        aff = small.tile([1, 6], F32, tag="aff")
        nc.scalar.activation(aff[:], afz[:], AF.Tanh)
        if debug:
            nc.sync.dma_start(dbg["dbg_aff"].ap(), aff[:])

        # affine node transform
        paf = psum_s.tile([33, 6], F32, tag="sps")
        nc.tensor.matmul(paf[:], ONES[0:1, 0:33], aff[:], start=True, stop=True)
        affb = small.tile([33, 6], F32, tag="affb")
        nc.scalar.copy(affb[:], paf[:])

    def affine_nodes(nodes_const, tag):
        n = nodes_p.tile([33, 2], F32, tag=tag)
        u = temps.tile([33, 1], F32, tag="affu")
        v = temps.tile([33, 1], F32, tag="affv")
        nc.vector.tensor_scalar_mul(u[:], nodes_const[:, 0:1], affb[:, 0:1])
        nc.vector.tensor_scalar_mul(v[:], nodes_const[:, 1:2], affb[:, 3:4])
        nc.vector.tensor_tensor(n[:, 0:1], u[:], v[:], ALU.add)
        nc.vector.tensor_scalar_mul(u[:], nodes_const[:, 0:1], affb[:, 1:2])
        nc.vector.tensor_scalar_mul(v[:], nodes_const[:, 1:2], affb[:, 4:5])
        nc.vector.tensor_tensor(n[:, 1:2], u[:], v[:], ALU.add)
        return n

    n1 = affine_nodes(NODES1, "n1_0")
    n2 = affine_nodes(NODES2, "n2_0")

    LYr = consts.tile([2, 128], F16, tag="lyr")
    nc.vector.tensor_copy(LYr[:], LY[:])

    # ---- renderer (setup / face-streaming split) --------------------------
    rend_scr = dram.tile([4, 96, 256], F16, tag="rend_scr")
    macc = {}      # rslot -> (tile, first_flag_list)

    def render_setup(nodes_t, rslot, eng='dve'):
        """Emit coefficient computation for one render; returns state."""
        rows = {}
        for nm, lhsT, G in (("v0x", nodes_t[:, 0:1], G0), ("v0y", nodes_t[:, 1:2], G0),
                            ("v1x", nodes_t[:, 0:1], G1), ("v1y", nodes_t[:, 1:2], G1)):
            pg = psum_s.tile([1, 96], F32, tag="sps")
            nc.tensor.matmul(pg[:], lhsT, G[:], start=True, stop=True)
            t = rendp.tile([1, 96], F32, tag=f"r_{nm}")
            nc.scalar.copy(t[:], pg[:])
            rows[nm] = t

        def op2(nm, i0, i1, op):
            t = rendp.tile([1, 96], F32, tag=f"r_{nm}")
            nc.vector.tensor_tensor(t[:], i0, i1, op)
            return t

        ex = op2("ex", rows["v1x"][:], rows["v0x"][:], ALU.subtract)
        ey = op2("ey", rows["v1y"][:], rows["v0y"][:], ALU.subtract)
        ex2 = op2("ex2", ex[:], ex[:], ALU.mult)
        ey2 = op2("ey2", ey[:], ey[:], ALU.mult)
        e2 = op2("e2", ex2[:], ey2[:], ALU.add)
        el = rendp.tile([1, 96], F32, tag="r_el")
        nc.scalar.activation(el[:], e2[:], AF.Sqrt)
        nc.vector.tensor_scalar_add(el[:], el[:], 1e-8)
        il = rendp.tile([1, 96], F32, tag="r_il")
        nc.vector.reciprocal(il[:], el[:])
        fx0 = rows["v0x"][0:1, 0::3]; fx1 = rows["v0x"][0:1, 1::3]; fx2 = rows["v0x"][0:1, 2::3]
        fy0 = rows["v0y"][0:1, 0::3]; fy1 = rows["v0y"][0:1, 1::3]; fy2 = rows["v0y"][0:1, 2::3]
        d10x = rendp.tile([1, 32], F32, tag="r_a1")
        nc.vector.tensor_tensor(d10x[:], fx1, fx0, ALU.subtract)
        d20y = rendp.tile([1, 32], F32, tag="r_a2")
        nc.vector.tensor_tensor(d20y[:], fy2, fy0, ALU.subtract)
        p1t = rendp.tile([1, 32], F32, tag="r_a3")
        nc.vector.tensor_tensor(p1t[:], d10x[:], d20y[:], ALU.mult)
        d10y = rendp.tile([1, 32], F32, tag="r_a4")
        nc.vector.tensor_tensor(d10y[:], fy1, fy0, ALU.subtract)
        d20x = rendp.tile([1, 32], F32, tag="r_a5")
        nc.vector.tensor_tensor(d20x[:], fx2, fx0, ALU.subtract)
        p2t = rendp.tile([1, 32], F32, tag="r_a6")
        nc.vector.tensor_tensor(p2t[:], d10y[:], d20x[:], ALU.mult)
        area = rendp.tile([1, 32], F32, tag="r_area")
        nc.vector.tensor_tensor(area[:], p1t[:], p2t[:], ALU.subtract)
        sg = rendp.tile([1, 32], F32, tag="r_sg")
        nc.scalar.activation(sg[:], area[:], AF.Sign)
        s96 = rendp.tile([1, 96], F32, tag="r_s96")
        for j in range(3):
            nc.vector.tensor_copy(s96[0:1, j::3], sg[:])
        m = rendp.tile([1, 96], F32, tag="r_m")
        nc.vector.tensor_tensor(m[:], s96[:], il[:], ALU.mult)
        nc.vector.tensor_scalar_mul(m[:], m[:], SHARP)
        mneg = rendp.tile([1, 96], F32, tag="r_mneg")
        nc.vector.tensor_scalar_mul(mneg[:], m[:], -1.0)
        acoef = op2("acoef", ey[:], mneg[:], ALU.mult)
        bcoef = op2("bcoef", ex[:], m[:], ALU.mult)
        cx = op2("cx", ey[:], rows["v0x"][:], ALU.mult)
        cy = op2("cy", ex[:], rows["v0y"][:], ALU.mult)
        cd = op2("cd", cx[:], cy[:], ALU.subtract)
        ccoef = op2("ccoef", cd[:], m[:], ALU.mult)
        pct = psum_s.tile([96, 3], F32, tag="sps")
        nc.tensor.transpose(pct[:, 0:1], acoef[:], IDENT[0:1, 0:1])
        nc.tensor.transpose(pct[:, 1:2], bcoef[:], IDENT[0:1, 0:1])
        nc.tensor.transpose(pct[:, 2:3], ccoef[:], IDENT[0:1, 0:1])
        acb = rendp.tile([96, 3], F32, tag="r_acb")
        nc.scalar.copy(acb[:], pct[:])
        RB = rendp.tile([96, 256], F16, tag="r_RB")
        nc.vector.tensor_scalar(RB[:, 0:128], XS[0:96, :], acb[:, 0:1],
                                acb[:, 2:3], ALU.mult, ALU.add)
        nc.vector.tensor_scalar(RB[:, 128:256], XS[0:96, :], 0.0,
                                acb[:, 1:2], ALU.mult, ALU.add)
        nc.sync.dma_start(rend_scr[rslot], RB[:])
        maccD = feat.tile([128, 128], F32, tag=f"maccD{rslot}", name=f"maccD{rslot}")
        gmin = feat.tile([128, 8, 128], F16, tag=f"gmin{rslot}", name=f"gmin{rslot}")
        macc[rslot] = [maccD, True]
        return {'rslot': rslot, 'grp': None, 'gmin': gmin, 'eng': eng}

    def render_faces(st, pD_pool, f0, f1):
        """Emit face matmuls + per-face min into gmin slots for faces
        [f0, f1); every 8th face collapses the slots into macc.
        eng='pair': two faces share one [128,768] psum tile and one DVE
        reduce (fewer PE<->DVE sync points mid-decoder)."""
        rslot = st['rslot']
        gmin = st['gmin']
        pair = st['eng'] == 'pair'
        scr = rend_scr[rslot].rearrange("e c -> (e c)")
        for fi in range(f0, f1):
            g = fi // 4
            if st['grp'] is None or st['grp'][0] != g:
                grp2 = grpp.tile([2, 1536], F16, tag="r_grp2")
                src = bass.AP(tensor=scr.tensor, offset=scr.offset + g * 12 * 256,
                              ap=[[128, 2], [256, 12], [1, 128]])
                nc.sync.dma_start(grp2.rearrange("p (e x) -> p e x", e=12), src)
                st['grp'] = (g, grp2)
            grp2 = st['grp'][1]
            fl = fi % 4
            if pair:
                if fi % 2 == 0:
                    pD = pD_pool.tile([128, 1024], F32, tag="rpD2")
                    st['pD'] = pD
                pD = st['pD']
                # faces at col 0 / 512 so each MM stays inside one PSUM bank
                nc.tensor.matmul(pD[:, (fi % 2) * 512:(fi % 2) * 512 + 384],
                                 LYr[:], grp2[:, ts(fl, 384)],
                                 start=True, stop=True)
                if fi % 2 == 1:
                    pDv = bass.AP(tensor=pD.tensor, offset=pD.offset,
                                  ap=[pD.ap[0], [512, 2], [1, 128], [128, 3]])
                    nc.vector.tensor_reduce(gmin[:, (fi % 8) - 1:(fi % 8) + 1, :],
                                            pDv, AX.X, ALU.min)
            else:
                pD = pD_pool.tile([128, 384], F32, tag="rpD")
                nc.tensor.matmul(pD[:], LYr[:], grp2[:, ts(fl, 384)],
                                 start=True, stop=True)
                pDv = bass.AP(tensor=pD.tensor, offset=pD.offset,
                              ap=[pD.ap[0], [1, 128], [128, 3]])
                nc.vector.tensor_reduce(gmin[:, fi % 8, :], pDv, AX.X, ALU.min)
            if fi % 8 == 7:
                mt, first = macc[rslot]
                gv = bass.AP(tensor=gmin.tensor, offset=gmin[:].offset,
                             ap=[gmin[:].ap[0], [1, 128], [128, 8]])
                if first:
                    nc.vector.tensor_reduce(mt[:], gv, AX.X, ALU.max)
                    macc[rslot][1] = False
                else:
                    cmax = temps.tile([128, 128], F32, tag="r_cmax")
                    nc.vector.tensor_reduce(cmax[:], gv, AX.X, ALU.max)
                    nc.vector.tensor_tensor(mt[:], mt[:], cmax[:], ALU.max)

    def render_finish(st, out_ch):
        rslot = st['rslot']
        mD = macc[rslot][0]
        soft = temps.tile([128, 128], F32, tag="r_soft")
        nc.scalar.activation(soft[:], mD[:], AF.Sigmoid)
        nc.sync.dma_start(out_d.ap()[out_ch], soft[:])

    st1 = render_setup(n1, 0, 'pair')
    st2 = render_setup(n2, 1, 'pair')

    # ---- decoder with interleaved renders --------------------------------
    with tc.tile_pool(name="psum_db", bufs=2, space="PSUM") as psum_db, \
         tc.tile_pool(name="psum_pd", bufs=2, space="PSUM") as psum_pd:
        # dw1: out (256, 16, 16); in = up4(4 blk) + f3(2 blk)
        for ocb in range(2):
            ws6 = [wslice() for _ in range(6)]
            ps_full = psum_db.tile([128, 512], F32, tag="bps")
            ps = ps_full[:, 0:256]
            psv = ps.rearrange("p (a b) -> p a b", a=16)
            first = True
            for gi in range(6):
                src = up4_pad[:, gi] if gi < 4 else f3_pad[:, gi - 4]
                for tap in range(25):
                    dy, dx = tap // 5, tap % 5
                    nc.tensor.matmul(psv, ws6[gi][:, tap, :],
                                     src[:, dy:dy + 16, dx:dx + 16],
                                     start=first, stop=(gi == 5 and tap == 24))
                    first = False
            nc.scalar.activation(
                u1_pad[:, ocb, 2:18, 2:18],
                ps[:], AF.Relu, bias=db1[:, ocb:ocb + 1], scale=1.0)
            render_faces(st1, psum_pd, 8 * ocb, 8 * ocb + 8)

        upsample2(u1_pad[:, :, 2:18, 2:18], u1up_pad[:, :, 2:34, 2:34], 128, 2, 16, 16)

        # dw2: out (128, 32, 32); in = u1up(2 blk) + f2(1 blk)
        dw2s = [wslice() for _ in range(3)]
        for c in range(2):
            ps = psum_db.tile([128, 512], F32, tag="bps")
            psv = ps.rearrange("p (a b) -> p a b", a=16)
            first = True
            for bi in range(3):
                src = u1up_pad[:, bi] if bi < 2 else f2_pad
                for tap in range(25):
                    dy, dx = tap // 5, tap % 5
                    nc.tensor.matmul(psv, dw2s[bi][:, tap, :],
                                     src[:, dy + 16 * c:dy + 16 * c + 16, dx:dx + 32],
                                     start=first, stop=(bi == 2 and tap == 24))
                    first = False
            nc.scalar.activation(
                u2_pad[:, 2 + 16 * c:18 + 16 * c, 2:34],
                ps[:], AF.Relu, bias=db2[:], scale=1.0)
            render_faces(st1, psum_pd, 16 + 8 * c, 24 + 8 * c)
        render_finish(st1, 0)

        u2v = u2_pad.rearrange("p (b y) x -> p b y x", b=1)
        u2upv = u2up_pad.rearrange("p (b y) x -> p b y x", b=1)
        upsample2(u2v[:, :, 2:34, 2:34], u2upv[:, :, 2:66, 2:66], 128, 1, 32, 32)

        # dw3: out (128=64x2, 64, 64); in = u2up(1 blk 128) + f1(64 pair-packed)
        dw3a = wslice()
        _lastsl = wslice().rearrange("p a b -> p (a b)")
        dw3bP = _lastsl[:, 0:15 * 128].rearrange("p (a b) -> p a b", a=15)
        dwoP = _lastsl[:, 15 * 128:15 * 128 + 60].rearrange(
            "p (a b) -> p a b", a=15)
        for c in range(8):
            ps = psum_db.tile([128, 512], F32, tag="bps")
            psv = ps.rearrange("p (a b) -> p a b", a=8)
            for tap in range(25):
                dy, dx = tap // 5, tap % 5
                nc.tensor.matmul(psv, dw3a[:, tap, :],
                                 u2up_pad[:, dy + 8 * c:dy + 8 * c + 8, dx:dx + 64],
                                 start=(tap == 0), stop=False)
            for t, (dy_lo, dx) in enumerate(PAIR_TAPS):
                nc.tensor.matmul(psv, dw3bP[:, t, :],
                                 f1_pad[:, dy_lo + 8 * c:dy_lo + 8 * c + 8, dx:dx + 64],
                                 start=False, stop=(t == 14))
            nc.scalar.activation(
                u3_pad[0:64, 2 + 8 * c:10 + 8 * c, 2:66],
                ps[0:64], AF.Relu, bias=db3[0:64], scale=1.0)
            nc.scalar.activation(
                u3_pad[64:128, 1 + 8 * c:9 + 8 * c, 2:66],
                ps[64:128], AF.Relu, bias=db3[64:128], scale=1.0)
            render_faces(st2, psum_pd, 4 * c, 4 * c + 4)
        render_finish(st2, 2)
        if debug:
            nc.sync.dma_start(dbg["dbg_u3"].ap(), u3_pad[:])

    # ---- dwo + disp -------------------------------------------------------
    dwo_scr = dram.tile([4, 64, 64], F16, tag="dwo_scr")
    dwo_f = dwo_scr.rearrange("c y x -> c (y x)")
    with tc.tile_pool(name="psum_o", bufs=2, space="PSUM") as psum_o, \
         tc.tile_pool(name="psum_u", bufs=2, space="PSUM") as psum_u, \
         tc.tile_pool(name="psum_p2", bufs=2, space="PSUM") as psum_p2:
        for c in range(8):
            ps = psum_o.tile([4, 512], F32, tag="ops")
            psv = ps.rearrange("p (a b) -> p a b", a=8)
            for t, (dy_lo, dx) in enumerate(PAIR_TAPS):
                nc.tensor.matmul(psv, dwoP[:, t, :],
                                 u3_pad[:, dy_lo + 8 * c:dy_lo + 8 * c + 8, dx:dx + 64],
                                 start=(t == 0), stop=(t == 14))
            dt_ = chunks.tile([4, 512], F16, tag="dwot")
            nc.scalar.activation(dt_[:], ps[:], AF.Tanh, bias=dbo[:], scale=1.0)
            nc.sync.dma_start(dwo_f[:, ts(c, 512)], dt_[:])

        # disp: repartition [4,64,64] -> [64, 4, 64], upsample-y via matmul,
        # upsample-x via strided vector ops -> disp_sb [128, 4, 128] fp16
        d64 = feat.tile([64, 4, 64], F16, tag="d64")
        src = bass.AP(tensor=dwo_scr.tensor, offset=dwo_scr.offset,
                      ap=[[64, 64], [4096, 4], [1, 64]])
        nc.sync.dma_start(d64[:], src)
        for ch in range(4):
            pu = psum_u.tile([128, 64], F32, tag="ups")
            nc.tensor.matmul(pu[:], U64F[:], d64[:, ch, :], start=True, stop=True)
            dch = disp_sb[:, ch, :]
            tb = temps.tile([128, 63], F32, tag="disptb")
            nc.vector.tensor_copy(dch[:, 0:1], pu[:, 0:1])
            nc.vector.tensor_scalar_mul(tb[:], pu[:, 0:63], 1.0 / 3.0)
            nc.vector.tensor_tensor(tb[:], tb[:], pu[:, 1:64], ALU.add)
            nc.vector.tensor_scalar_mul(dch[:, 2:127:2], tb[:], 0.75)
            nc.vector.tensor_scalar_mul(tb[:], pu[:, 1:64], 1.0 / 3.0)
            nc.vector.tensor_tensor(tb[:], tb[:], pu[:, 0:63], ALU.add)
            nc.vector.tensor_scalar_mul(dch[:, 1:126:2], tb[:], 0.75)
            nc.vector.tensor_copy(dch[:, 127:128], pu[:, 63:64])
        if debug:
            nc.sync.dma_start(dbg["dbg_disp"].ap(), disp_sb[:])

        # ---- deformation iterations --------------------------------------
        # bilinear hat weights via ACT: w = relu(1 - |affine(coord)|), with
        # the affine folded into the Abs scale/bias (frees the DVE)
        def sample_prep(nodes_t, tag):
            tp = psum_s.tile([1, 33], F32, tag="sps")
            nc.tensor.transpose(tp[:], nodes_t[:, 1:2], IDENT[0:33, 0:33])
            ytr = small.tile([1, 33], F32, tag=f"ytr{tag}")
            nc.scalar.copy(ytr[:], tp[:])
            ybc = psum_s.tile([128, 33], F32, tag="sps")
            nc.tensor.matmul(ybc[:], ONES[:], ytr[:], start=True, stop=True)
            aw = small.tile([128, 33], F16, tag=f"aw{tag}")
            nc.scalar.activation(aw[:], ybc[:], AF.Abs, bias=NIOTAY2[:],
                                 scale=-64.0)
            wy = small.tile([128, 33], F16, tag=f"wy{tag}")
            nc.scalar.activation(wy[:], aw[:], AF.Relu, bias=ONECOL[:],
                                 scale=-1.0)
            xcb = small.tile([33, 1], F32, tag=f"xcb{tag}")
            nc.vector.tensor_scalar(xcb[:], nodes_t[:, 0:1], -64.0, -63.5,
                                    ALU.mult, ALU.add)
            awx = small.tile([33, 128], F16, tag=f"awx{tag}")
            nc.scalar.activation(awx[:], IOTAX33[:], AF.Abs, bias=xcb[:],
                                 scale=1.0)
            wx = small.tile([33, 128], F16, tag=f"wx{tag}")
            nc.scalar.activation(wx[:], awx[:], AF.Relu, bias=ONECOL[0:33],
                                 scale=-1.0)
            return wy, wx

        def sample_all(wy, wx, tag):
            pssm = psum_s.tile([33, 512], F32, tag="sps")
            nc.tensor.matmul(pssm[:], wy[:],
                             disp_sb.rearrange("p c x -> p (c x)"),
                             start=True, stop=True)
            prod = temps.tile([33, 4, 128], F32, tag="sp")
            wx_b = bass.AP(tensor=wx.tensor, offset=wx[:].offset,
                           ap=[wx[:].ap[0], [0, 4], [1, 128]])
            nc.vector.tensor_tensor(prod[:], pssm.rearrange("p (c x) -> p c x", c=4),
                                    wx_b, ALU.mult)
            dP = small.tile([33, 4], F32, tag=f"dP{tag}")
            nc.vector.tensor_reduce(dP[:], prod[:], AX.X, ALU.add)
            return dP

        # interleave the two independent deformation chains
        for it in range(ITER):
            wy1, wx1 = sample_prep(n1, "c1")
            wy2, wx2 = sample_prep(n2, "c2")
            dP1 = sample_all(wy1, wx1, "s1")
            dP2 = sample_all(wy2, wx2, "s2")
            n1n = nodes_p.tile([33, 2], F32, tag=f"n1_{it + 1}")
            nc.vector.tensor_tensor(n1n[:, 0:1], n1[:, 0:1], dP1[:, 0:1], ALU.add)
            nc.vector.tensor_tensor(n1n[:, 1:2], n1[:, 1:2], dP1[:, 1:2], ALU.subtract)
            n1 = n1n
            n2n = nodes_p.tile([33, 2], F32, tag=f"n2_{it + 1}")
            t2a = temps.tile([33, 2], F32, tag="t2a")
            t2b = temps.tile([33, 2], F32, tag="t2b")
            nc.vector.tensor_scalar_mul(t2a[:], dP2[:, 2:4], W2M[:])
            nc.vector.tensor_scalar_mul(t2b[:], dP2[:, 0:2], W0M[:])
            nc.vector.tensor_tensor(t2a[:], t2a[:], t2b[:], ALU.add)
            nc.vector.tensor_tensor(n2n[:, 0:1], n2[:, 0:1], t2a[:, 0:1], ALU.add)
            nc.vector.tensor_tensor(n2n[:, 1:2], n2[:, 1:2], t2a[:, 1:2], ALU.subtract)
            n2 = n2n

        if debug:
            nc.sync.dma_start(dbg["dbg_n1"].ap(), n1[:])
            nc.sync.dma_start(dbg["dbg_n2"].ap(), n2[:])

        # final renders, interleaved, with GpSimd offload
        st3 = render_setup(n1, 2, 'dve')
        st4 = render_setup(n2, 3, 'dve')
        for k in range(8):
            render_faces(st3, psum_p2, 4 * k, 4 * k + 4)
            render_faces(st4, psum_p2, 4 * k, 4 * k + 4)
        render_finish(st3, 1)
        render_finish(st4, 3)

    ctx.close()


# ---------------------------------------------------------------------------
# public entry point
# ---------------------------------------------------------------------------

def _get_program(debug=False):
    key = ("prog", debug)
    if key not in _CACHE:
        _CACHE[key] = _build_program(debug)
    return _CACHE[key]


def make_in_maps(host, img):
    in_maps = []
    for c in range(N_CORES):
        m = dict(host)
        m["img"] = img[c, 0]
        in_maps.append(m)
    return in_maps


def kernel(**inputs):
    from concourse import bass_utils

    nc = _get_program(debug=_CACHE.get("debug_mode", False))
    if "host" not in _CACHE:
        _CACHE["host"] = _prep_host(inputs)
    host = _CACHE["host"]

    img = np.asarray(inputs["img"], np.float32)   # (8, 1, 128, 128)
    in_maps = make_in_maps(host, img)
    res = bass_utils.run_bass_kernel_spmd(nc, in_maps, core_ids=list(range(N_CORES)))
    _CACHE["last_results"] = res
    out = np.stack([res.results[c]["out"] for c in range(N_CORES)], 0)
    return out.astype(np.float32)

